# revision 5
# baseline (speedup 1.0000x reference)
"""MultiHeadCrossAttention kernel for 8 Trainium2 NeuronCores.

Problem (hardcoded): B=4, Sx=Sy=1024, DIM=1024, H=16, Dh=64, fp32.
  Q = x@W_Qx.T+b_Qx ; K = cat(x@W_Kx.T+b_Kx, y@W_Ky.T+b_Ky) per head
  V = cat(x@W_Vx.T+b_Vx, y@W_Vy.T+b_Vy) ; out = softmax(QK^T/8)V @ W_out.T + b_out

Sharding: core c -> (batch b = c//2, head-group g = c%2 of 8 heads).
Each core computes its batch's attention for its 8 heads plus the partial
out-projection over its 512 features; host sums the two partials per batch
and adds b_out (the "all-reduce after to_out", done in the gather).

Device layout choices (all matmuls natural, zero on-device transposes):
 - activations pre-transposed on host: xT/yT [dim, seq]
 - Q/K projections in transposed domain [feat, seq]  (bias = per-partition)
 - V in natural domain [seq, feat] with host-broadcast bias, plus a ones
   column per head -> AV matmul row 64 yields the softmax denominator
 - scoresT [k, q] via lhsT=KT (d=64 contraction; head pairs row-pack the PE)
 - exp on ACT only (no max subtraction: |scores| <~ 3), normalize via
   PE-broadcast reciprocal, out-projection in transposed domain [m, s]

v2 scheduling: DMA loads split into priority slices and issued across 4
engine queues (first V matmul ~4us instead of 13); reciprocal via the
fast approx custom-DVE op; out-projection fully 3-deep pipelined right
after the last AV matmul; fp16 partial outputs (halves the output DMA).
"""

import os
import sys

os.environ.setdefault("MYCRO_LOCAL_CACHE", "1")
if "/opt/trn_rl_repo" not in sys.path:
    sys.path.insert(0, "/opt/trn_rl_repo")

import ml_dtypes
import numpy as np

import concourse.bass as bass
import concourse.mybir as mybir
import concourse.tile as tile
from concourse import bass_utils
from concourse.bass_utils import run_bass_kernel_spmd

FP32 = mybir.dt.float32
FP16 = mybir.dt.float16
BF16 = mybir.dt.bfloat16

DIM = 1024
H = 16          # total heads
HG = 8          # heads per core (head-group)
DH = 64
S = 1024        # Sx = Sy
FS = 512        # feature slice per core (HG * DH)
NCORES = 8

# ---------------------------------------------------------------------------
# harness patches (this snapshot's Tile emits >1 wait per instruction in a
# few places; HW instructions hold one wait)
# ---------------------------------------------------------------------------

def _patched_drain_and_barrier(self, tick_clock, wait_clock):
    from bass_rust import ScopedClock

    nc = self.nc
    drain_inst = nc.sync.drain()
    wait_clock.add_sem_waits(
        drain_inst.ins, ScopedClock({None: tick_clock.global_clock})
    )
    si = drain_inst.ins.sync_info
    waits = list(si.on_wait)
    if len(waits) > 1:
        del si.on_wait[1:]
        for w in waits[1:]:
            nop = nc.sync.nop(nofuse=True, hint="drain_wait_spill")
            if nop.ins.sync_info is None:
                nop.ins.sync_info = mybir.SyncInfo(on_wait=[], on_update=[])
            nop.ins.sync_info.on_wait.append(w)

    nc.all_engine_barrier()
    assert self.sems is not None
    popped = nc._tile_sem_poison_stack.pop()
    assert popped is self._sem_poison
    nc.clear_and_free_semaphores(list(self.sems.allocated().values()))
    nc.all_engine_barrier()


def _spill_excess_waits(nc):
    n = 0
    for fn in nc.m.functions:
        for bb in fn.blocks:
            new_insts = []
            for inst in bb.instructions:
                si = getattr(inst, "sync_info", None)
                cap = 2 if isinstance(inst, mybir.InstEventSemaphore) else 1
                if si is not None and si.on_wait and len(si.on_wait) > cap:
                    extras = list(si.on_wait[cap:])
                    del si.on_wait[cap:]
                    for w in extras:
                        new_insts.append(
                            mybir.InstNoOp(
                                name=f"wspill-{nc.next_id()}",
                                engine=inst.engine,
                                ins=[],
                                outs=[],
                                sync_info=mybir.SyncInfo(on_wait=[w], on_update=[]),
                            )
                        )
                        n += 1
                new_insts.append(inst)
            bb.instructions[:] = new_insts
    return n


tile.TileContext._drain_and_barrier = _patched_drain_and_barrier

bass_utils.upload_artifacts = lambda tmpdir: tmpdir  # no S3 in container


def _register_ntff_hook():
    """Best-effort: enables trace=True runs (used by test harness only)."""
    try:
        try:
            from antenv.axon_hooks import set_axon_ntff_profile_hook
        except ImportError:
            # this container's antenv lacks axon_hooks — synthesize it
            import types

            import antenv

            mod = types.ModuleType("antenv.axon_hooks")
            _hook = [None]
            mod.set_axon_ntff_profile_hook = lambda h: _hook.__setitem__(0, h)
            mod.get_axon_ntff_profile_hook = lambda: _hook[0]
            sys.modules["antenv.axon_hooks"] = mod
            antenv.axon_hooks = mod
            set_axon_ntff_profile_hook = mod.set_axon_ntff_profile_hook
        sys.path.insert(0, "/root/.axon_site")
        from trn_agent_boot.trn_boot import _ntff_profile_via_ctypes

        set_axon_ntff_profile_hook(
            _ntff_profile_via_ctypes("/opt/axon/libaxon_pjrt.so")
        )
    except Exception:
        pass


# ---------------------------------------------------------------------------
# device program (identical on all 8 cores; per-core data differs)
# ---------------------------------------------------------------------------

def _build_program():
    nc = bass.Bass()

    xT = nc.declare_dram_parameter("xT", [DIM, S], BF16, isOutput=False)
    yT = nc.declare_dram_parameter("yT", [DIM, S], BF16, isOutput=False)
    wq = nc.declare_dram_parameter("wq", [DIM, FS], BF16, isOutput=False)
    wkx = nc.declare_dram_parameter("wkx", [DIM, FS], BF16, isOutput=False)
    wky = nc.declare_dram_parameter("wky", [DIM, FS], BF16, isOutput=False)
    wvx = nc.declare_dram_parameter("wvx", [DIM, FS], BF16, isOutput=False)
    wvy = nc.declare_dram_parameter("wvy", [DIM, FS], BF16, isOutput=False)
    wo = nc.declare_dram_parameter("wo", [FS, DIM], BF16, isOutput=False)
    bq = nc.declare_dram_parameter("bq", [128, 4], FP32, isOutput=False)
    bkx = nc.declare_dram_parameter("bkx", [128, 4], FP32, isOutput=False)
    bky = nc.declare_dram_parameter("bky", [128, 4], FP32, isOutput=False)
    bvx_bc = nc.declare_dram_parameter("bvx_bc", [1, FS], FP32, isOutput=False)
    bvy_bc = nc.declare_dram_parameter("bvy_bc", [1, FS], FP32, isOutput=False)
    outT = nc.declare_dram_parameter("outT", [DIM, S], FP16, isOutput=True)

    EXP = mybir.ActivationFunctionType.Exp

    with tile.TileContext(nc) as tc:
        import contextlib

        with contextlib.ExitStack() as ctx:
            big = ctx.enter_context(tc.tile_pool(name="big", bufs=24))
            wpool = ctx.enter_context(tc.tile_pool(name="wpool", bufs=40))
            qkv = ctx.enter_context(tc.tile_pool(name="qkv", bufs=12))
            vpool = ctx.enter_context(tc.tile_pool(name="vpool", bufs=16))
            ppool = ctx.enter_context(tc.tile_pool(name="ppool", bufs=6))
            opool = ctx.enter_context(tc.tile_pool(name="opool", bufs=2))
            spool = ctx.enter_context(tc.tile_pool(name="spool", bufs=4))
            cpool = ctx.enter_context(tc.tile_pool(name="cpool", bufs=1))
            dpool = ctx.enter_context(tc.tile_pool(name="dpool", bufs=8, space="DRAM"))
            mm_ps = ctx.enter_context(tc.tile_pool(name="mm_ps", bufs=3, space="PSUM"))
            ot_ps = ctx.enter_context(tc.tile_pool(name="ot_ps", bufs=2, space="PSUM"))

            # ---- constants ----
            ones_f32 = cpool.tile([128, 64], FP32, tag="ones_f32")
            nc.vector.memset(ones_f32[:, :], 1.0)
            bq_sb = cpool.tile([128, 4], FP32, tag="bq")
            bkx_sb = cpool.tile([128, 4], FP32, tag="bkx")
            bky_sb = cpool.tile([128, 4], FP32, tag="bky")
            bvx_sb = cpool.tile([128, FS], FP32, tag="bvx")
            bvy_sb = cpool.tile([128, FS], FP32, tag="bvy")

            def _bcast_ap(h):
                return bass.AP(
                    tensor=h[:, :].tensor, offset=h[:, :].offset,
                    ap=[[0, 128]] + [list(a) for a in h[:, :].ap[1:]],
                )

            # ---- tile allocations for activations + weights ----
            xt = [big.tile([128, S], BF16, tag="big", name=f"xt{i}") for i in range(8)]
            yt = [big.tile([128, S], BF16, tag="big", name=f"yt{i}") for i in range(8)]
            wo_sb = [big.tile([128, S], BF16, tag="big", name=f"wo{i}") for i in range(4)]
            wvx_sb = [wpool.tile([128, FS], BF16, tag="w", name=f"wvx{i}") for i in range(8)]
            wvy_sb = [wpool.tile([128, FS], BF16, tag="w", name=f"wvy{i}") for i in range(8)]
            qk_w = [[wpool.tile([128, FS], BF16, tag="w", name=f"wp{pi}_{ct}")
                     for ct in range(8)] for pi in range(3)]  # Q, Kx, Ky

            # ---- DMA issues: priority order, spread across the 3 engine
            # queues that can issue DMAs (sync/SP, scalar/ACT, gpsimd).
            # sync:   xt column-slices (V-x sg0 needs [0:256] of every chunk)
            # scalar: wvx, wq, wkx  (scalar is idle until the first exp)
            # gpsimd: biases, wvy, yt halves, wky, wo
            for ct in range(8):
                nc.sync.dma_start(out=xt[ct][:, 0:256], in_=xT[ct * 128:(ct + 1) * 128, 0:256])
            for ct in range(8):
                nc.sync.dma_start(out=xt[ct][:, 256:512], in_=xT[ct * 128:(ct + 1) * 128, 256:512])
            for ct in range(8):
                nc.sync.dma_start(out=xt[ct][:, 512:1024], in_=xT[ct * 128:(ct + 1) * 128, 512:1024])

            for ct in range(8):
                nc.scalar.dma_start(out=wvx_sb[ct], in_=wvx[ct * 128:(ct + 1) * 128, :])
            for ct in range(8):
                nc.scalar.dma_start(out=qk_w[0][ct], in_=wq[ct * 128:(ct + 1) * 128, :])
            for ct in range(8):
                nc.scalar.dma_start(out=qk_w[1][ct], in_=wkx[ct * 128:(ct + 1) * 128, :])

            nc.gpsimd.dma_start(out=bq_sb, in_=bq[:, :])
            nc.gpsimd.dma_start(out=bkx_sb, in_=bkx[:, :])
            nc.gpsimd.dma_start(out=bky_sb, in_=bky[:, :])
            nc.gpsimd.dma_start(out=bvx_sb, in_=_bcast_ap(bvx_bc))
            nc.gpsimd.dma_start(out=bvy_sb, in_=_bcast_ap(bvy_bc))
            for ct in range(8):
                nc.gpsimd.dma_start(out=wvy_sb[ct], in_=wvy[ct * 128:(ct + 1) * 128, :])
            for ct in range(8):
                nc.gpsimd.dma_start(out=yt[ct][:, 0:512], in_=yT[ct * 128:(ct + 1) * 128, 0:512])
            for ct in range(8):
                nc.gpsimd.dma_start(out=yt[ct][:, 512:1024], in_=yT[ct * 128:(ct + 1) * 128, 512:1024])
            for ct in range(8):
                nc.gpsimd.dma_start(out=qk_w[2][ct], in_=wky[ct * 128:(ct + 1) * 128, :])
            for ft in range(4):
                nc.gpsimd.dma_start(out=wo_sb[ft], in_=wo[ft * 128:(ft + 1) * 128, :])

            # ---- V projections (natural domain, bias + ones column) ----
            V = [vpool.tile([128, HG, DH + 1], BF16, tag="v", name=f"V{i}") for i in range(16)]
            for src_is_y in (False, True):
                bias_sb = bvy_sb if src_is_y else bvx_sb
                base = 8 if src_is_y else 0
                w_sb = wvy_sb if src_is_y else wvx_sb
                act = yt if src_is_y else xt
                for sg in range(4):  # two s-tiles per psum group
                    ps = mm_ps.tile([128, 1024], FP32, tag="mm", name="vps")
                    for ct in range(8):
                        for half in range(2):
                            st = 2 * sg + half
                            nc.tensor.matmul(
                                ps[:, half * 512:(half + 1) * 512],
                                act[ct][:, st * 128:(st + 1) * 128],
                                w_sb[ct][:, :],
                                start=(ct == 0),
                                stop=(ct == 7),
                            )
                    for half in range(2):
                        st = 2 * sg + half
                        vt = V[base + st]
                        nc.vector.tensor_add(
                            out=vt[:, :, 0:DH],
                            in0=ps[:, half * 512:(half + 1) * 512].rearrange(
                                "p (h d) -> p h d", h=HG),
                            in1=bias_sb[:, :].rearrange("p (h d) -> p h d", h=HG),
                        )
                        nc.vector.tensor_copy(
                            out=vt[:, :, DH:DH + 1],
                            in_=ones_f32[:, 0:HG].rearrange("p (h o) -> p h o", o=1),
                        )

            # ---- Q/K projections (transposed domain [feat, seq]) ----
            QT = [qkv.tile([128, S], BF16, tag="qkv", name=f"QT{i}") for i in range(4)]
            KxT = [qkv.tile([128, S], BF16, tag="qkv", name=f"KxT{i}") for i in range(4)]
            KyT = [qkv.tile([128, S], BF16, tag="qkv", name=f"KyT{i}") for i in range(4)]

            qk_act = [xt, xt, yt]
            qk_bias = [bq_sb, bkx_sb, bky_sb]
            qk_dst = [QT, KxT, KyT]
            qk_ps = {}

            def emit_qk_half(pi, ft, half):
                key = (pi, ft)
                if key not in qk_ps:
                    qk_ps[key] = mm_ps.tile(
                        [128, 1024], FP32, tag="mm", name=f"qkps{pi}_{ft}"
                    )
                ps = qk_ps[key]
                w_sb = qk_w[pi]
                act = qk_act[pi]
                for ct in (range(0, 4) if half == 0 else range(4, 8)):
                    for h2 in range(2):
                        nc.tensor.matmul(
                            ps[:, h2 * 512:(h2 + 1) * 512],
                            w_sb[ct][:, ft * 128:(ft + 1) * 128],
                            act[ct][:, h2 * 512:(h2 + 1) * 512],
                            start=(ct == 0),
                            stop=(ct == 7),
                        )
                if half == 1:
                    nc.vector.tensor_scalar_add(
                        out=qk_dst[pi][ft][:, :],
                        in0=ps[:, :],
                        scalar1=qk_bias[pi][:, ft:ft + 1],
                    )
                    del qk_ps[key]

            # upfront: ft=0 for all projections, plus all fts of proj 2 (wky)
            for pi in range(3):
                emit_qk_half(pi, 0, 0)
                emit_qk_half(pi, 0, 1)
            for ft in range(1, 4):
                emit_qk_half(2, ft, 0)
                emit_qk_half(2, ft, 1)

            # remaining 6 groups ride the attention phase's spare PE cycles
            qk_fillers = {(t, qt): (qt, t + 1) for t in range(3) for qt in range(2)}

            # ---- attention (head pairs row-pack; both q-tiles share one
            #      psum tile so exp runs once per (kt, head)) ----
            oT = [big.tile([128, S], BF16, tag="big", name=f"oT{i}") for i in range(4)]

            def emit_finalize(t, qt, o_sb, recips):
                for hh in range(2):
                        i = hh
                        rd = recips[i]
                        bc_sb = spool.tile([DH, 512], FP32, tag="bc", name="bc_sb")
                        rd_bcast = bass.AP(
                            tensor=rd.tensor, offset=rd.offset,
                            ap=[[0, DH]] + [list(a) for a in rd.ap[1:]],
                        )
                        nc.gpsimd.dma_start(out=bc_sb[:, :], in_=rd_bcast)
                        nc.vector.tensor_mul(
                            out=oT[t][hh * 64:hh * 64 + DH, qt * 512:(qt + 1) * 512],
                            in0=o_sb[i][:, :],
                            in1=bc_sb[:, :],
                        )

            pending = None
            for t in range(4):  # heads 2t, 2t+1
                for qt in range(2):
                    o_ps = [ot_ps.tile([128, 512], FP32, tag="ot", name=f"ops{i}")
                            for i in range(2)]  # per head of the pair
                    prev = None
                    for kt in range(16):
                        KT = KxT[t] if kt < 8 else KyT[t]
                        ks = (kt % 8) * 128
                        sc = mm_ps.tile([128, 1024], FP32, tag="mm", name="sc")
                        for hh in range(2):
                            nc.tensor.matmul(
                                sc[:, hh * 512:(hh + 1) * 512],
                                KT[hh * 64:(hh + 1) * 64, ks:ks + 128],
                                QT[t][hh * 64:(hh + 1) * 64, qt * 512:(qt + 1) * 512],
                                start=True,
                                stop=True,
                            )
                        p2 = ppool.tile([128, 1024], BF16, tag="p", name="p")
                        nc.scalar.activation(out=p2[:, :], in_=sc[:, :], func=EXP)
                        if (t, qt) in qk_fillers and kt in (4, 11):
                            fpi, fft = qk_fillers[(t, qt)]
                            emit_qk_half(fpi, fft, 0 if kt == 4 else 1)
                        if prev is not None:
                            for hh in range(2):
                                nc.tensor.matmul(
                                    o_ps[hh][0:DH + 1, :],
                                    V[kt - 1][:, 2 * t + hh, :],
                                    prev[:, hh * 512:(hh + 1) * 512],
                                    start=(kt == 1),
                                    stop=False,
                                )
                        prev = p2
                    for hh in range(2):
                        nc.tensor.matmul(
                            o_ps[hh][0:DH + 1, :],
                            V[15][:, 2 * t + hh, :],
                            prev[:, hh * 512:(hh + 1) * 512],
                            start=False,
                            stop=True,
                        )
                    if pending is not None:
                        emit_finalize(*pending)
                    o_sb = []
                    s2 = spool.tile([33, 512], FP32, tag="s2", name="s2")
                    for i in range(2):
                        nc.vector.tensor_copy(
                            out=s2[32 * i:32 * i + 1, :], in_=o_ps[i][DH:DH + 1, :]
                        )
                        ob = spool.tile([DH, 512], FP32, tag="osb", name="osb")
                        nc.vector.tensor_copy(out=ob[:, :], in_=o_ps[i][0:DH, :])
                        o_sb.append(ob)
                    rf2 = spool.tile([33, 512], FP32, tag="recipf", name="rf2")
                    nc.vector.reciprocal_approx_fast(out=rf2[:, :], in_=s2[:, :])
                    recips = []
                    for i in range(2):
                        rd = dpool.tile([1, 512], FP32, name="rd")
                        nc.gpsimd.dma_start(out=rd[:, :], in_=rf2[32 * i:32 * i + 1, :])
                        recips.append(rd)
                    pending = (t, qt, o_sb, recips)
            emit_finalize(*pending)

            # ---- out-projection (transposed domain [m, s]) ----
            # 3-deep software pipeline over the 8 m-tiles; ft0-2 first, ft3
            # (gated on the last head-pair's finalize) as late as possible.
            def op_mms(ps, mt, fts):
                for ft in fts:
                    for half in range(2):
                        nc.tensor.matmul(
                            ps[:, half * 512:(half + 1) * 512],
                            wo_sb[ft][:, mt * 128:(mt + 1) * 128],
                            oT[ft][:, half * 512:(half + 1) * 512],
                            start=(ft == 0),
                            stop=(ft == 3),
                        )

            def op_finish(ps, mt):
                osb = opool.tile([128, 1024], FP16, tag="osb2", name="osb2")
                nc.vector.tensor_copy(out=osb[:, :], in_=ps[:, :])
                nc.sync.dma_start(
                    out=outT[mt * 128:(mt + 1) * 128, :],
                    in_=osb[:, :],
                )

            op_tiles = {}
            for mt in range(3):
                op_tiles[mt] = mm_ps.tile([128, 1024], FP32, tag="mm", name=f"obs{mt}")
                op_mms(op_tiles[mt], mt, range(3))
            for mt in range(8):
                op_mms(op_tiles[mt], mt, [3])
                op_finish(op_tiles[mt], mt)
                nxt = mt + 3
                if nxt < 8:
                    op_tiles[nxt] = mm_ps.tile([128, 1024], FP32, tag="mm", name=f"obs{nxt}")
                    op_mms(op_tiles[nxt], nxt, range(3))

    # populate .instr bytes for extended-inst InstISA subclasses (the
    # custom-DVE reciprocal) — raw Bass skips this pass and the NEFF
    # compiler errors with "ISA wrong length" without it
    mybir.codegen_inst_isa_subclasses(nc)
    _spill_excess_waits(nc)
    return nc


_NC = None


def _get_program():
    global _NC
    if _NC is None:
        _NC = _build_program()
    return _NC


# ---------------------------------------------------------------------------
# host wrapper
# ---------------------------------------------------------------------------

def _prep_in_maps(x, y, W_Kx, b_Kx, W_Qx, b_Qx, W_Vx, b_Vx, W_Ky, b_Ky,
                  W_Vy, b_Vy, W_out, b_out):
    f32 = np.float32
    bf16 = ml_dtypes.bfloat16
    in_maps = []
    for c in range(NCORES):
        b = c // 2
        g = c % 2
        gs = slice(FS * g, FS * (g + 1))
        m = {
            "xT": np.ascontiguousarray(np.asarray(x[b], f32).T).astype(bf16),
            "yT": np.ascontiguousarray(np.asarray(y[b], f32).T).astype(bf16),
            "wq": np.ascontiguousarray((np.asarray(W_Qx, f32)[gs, :] / 8.0).T).astype(bf16),
            "wkx": np.ascontiguousarray(np.asarray(W_Kx, f32)[gs, :].T).astype(bf16),
            "wky": np.ascontiguousarray(np.asarray(W_Ky, f32)[gs, :].T).astype(bf16),
            "wvx": np.ascontiguousarray(np.asarray(W_Vx, f32)[gs, :].T).astype(bf16),
            "wvy": np.ascontiguousarray(np.asarray(W_Vy, f32)[gs, :].T).astype(bf16),
            "wo": np.ascontiguousarray(np.asarray(W_out, f32)[:, gs].T).astype(bf16),
            "bq": np.ascontiguousarray(
                (np.asarray(b_Qx, f32)[gs] / 8.0).reshape(4, 128).T),
            "bkx": np.ascontiguousarray(np.asarray(b_Kx, f32)[gs].reshape(4, 128).T),
            "bky": np.ascontiguousarray(np.asarray(b_Ky, f32)[gs].reshape(4, 128).T),
            "bvx_bc": np.ascontiguousarray(np.asarray(b_Vx, f32)[gs].reshape(1, FS)),
            "bvy_bc": np.ascontiguousarray(np.asarray(b_Vy, f32)[gs].reshape(1, FS)),
        }
        in_maps.append(m)
    return in_maps


def _assemble(results, b_out):
    B = 4
    out = np.empty((B, S, DIM), np.float32)
    bo = np.asarray(b_out, np.float32)
    for b in range(B):
        acc = (results[2 * b]["outT"].astype(np.float32)
               + results[2 * b + 1]["outT"].astype(np.float32))
        out[b] = acc.T + bo
    return out


def kernel(**inputs):
    nc = _get_program()
    in_maps = _prep_in_maps(**inputs)
    last_err = None
    for _attempt in range(3):
        try:
            res = run_bass_kernel_spmd(nc, in_maps, core_ids=list(range(NCORES)))
            return _assemble(res.results, inputs["b_out"])
        except Exception as e:  # transient NRT_EXEC_UNIT_UNRECOVERABLE after fresh compile
            last_err = e
            import time as _time
            _time.sleep(2.0)
    raise last_err


def kernel_traced(trace_cores=None, **inputs):
    """Same as kernel() but returns (out, BassKernelResults) with NTFF trace."""
    _register_ntff_hook()
    nc = _get_program()
    in_maps = _prep_in_maps(**inputs)
    res = run_bass_kernel_spmd(
        nc, in_maps, core_ids=list(range(NCORES)), trace=True,
        trace_cores=trace_cores or [0],
    )
    return _assemble(res.results, inputs["b_out"]), res


# revision 14
# speedup vs baseline: 1.0652x; 1.0652x over previous
"""MultiHeadCrossAttention kernel for 8 Trainium2 NeuronCores.

Problem (hardcoded): B=4, Sx=Sy=1024, DIM=1024, H=16, Dh=64, fp32.
  Q = x@W_Qx.T+b_Qx ; K = cat(x@W_Kx.T+b_Kx, y@W_Ky.T+b_Ky) per head
  V = cat(x@W_Vx.T+b_Vx, y@W_Vy.T+b_Vy) ; out = softmax(QK^T/8)V @ W_out.T + b_out

Sharding: core c -> (batch b = c//2, head-group g = c%2 of 8 heads).
Each core computes its batch's attention for its 8 heads plus the partial
out-projection over its 512 features; host sums the two partials per batch
and adds b_out (the "all-reduce after to_out", done in the gather).

Device layout choices (all matmuls natural, zero on-device transposes):
 - activations pre-transposed on host: xT/yT [dim, seq]
 - Q/K projections in transposed domain [feat, seq]  (bias = per-partition)
 - V in natural domain [seq, feat] with host-broadcast bias, plus a ones
   column per head -> AV matmul row 64 yields the softmax denominator
 - scoresT [k, q] via lhsT=KT (d=64 contraction; head pairs row-pack the PE)
 - exp on ACT only (no max subtraction: |scores| <~ 3), normalize via
   PE-broadcast reciprocal, out-projection in transposed domain [m, s]

v2 scheduling: DMA loads split into priority slices and issued across 4
engine queues (first V matmul ~4us instead of 13); reciprocal via the
fast approx custom-DVE op; out-projection fully 3-deep pipelined right
after the last AV matmul; fp16 partial outputs (halves the output DMA).
"""

import os
import sys

os.environ.setdefault("MYCRO_LOCAL_CACHE", "1")
if "/opt/trn_rl_repo" not in sys.path:
    sys.path.insert(0, "/opt/trn_rl_repo")

import ml_dtypes
import numpy as np

import concourse.bass as bass
import concourse.mybir as mybir
import concourse.tile as tile
from concourse import bass_utils
from concourse.bass_utils import run_bass_kernel_spmd

FP32 = mybir.dt.float32
FP16 = mybir.dt.float16
BF16 = mybir.dt.bfloat16

DIM = 1024
H = 16          # total heads
HG = 8          # heads per core (head-group)
DH = 64
S = 1024        # Sx = Sy
FS = 512        # feature slice per core (HG * DH)
NCORES = 8

# ---------------------------------------------------------------------------
# harness patches (this snapshot's Tile emits >1 wait per instruction in a
# few places; HW instructions hold one wait)
# ---------------------------------------------------------------------------

def _patched_drain_and_barrier(self, tick_clock, wait_clock):
    from bass_rust import ScopedClock

    nc = self.nc
    drain_inst = nc.sync.drain()
    wait_clock.add_sem_waits(
        drain_inst.ins, ScopedClock({None: tick_clock.global_clock})
    )
    si = drain_inst.ins.sync_info
    waits = list(si.on_wait)
    if len(waits) > 1:
        del si.on_wait[1:]
        for w in waits[1:]:
            nop = nc.sync.nop(nofuse=True, hint="drain_wait_spill")
            if nop.ins.sync_info is None:
                nop.ins.sync_info = mybir.SyncInfo(on_wait=[], on_update=[])
            nop.ins.sync_info.on_wait.append(w)

    nc.all_engine_barrier()
    assert self.sems is not None
    popped = nc._tile_sem_poison_stack.pop()
    assert popped is self._sem_poison
    nc.clear_and_free_semaphores(list(self.sems.allocated().values()))
    nc.all_engine_barrier()


def _spill_excess_waits(nc):
    n = 0
    for fn in nc.m.functions:
        for bb in fn.blocks:
            new_insts = []
            for inst in bb.instructions:
                si = getattr(inst, "sync_info", None)
                cap = 2 if isinstance(inst, mybir.InstEventSemaphore) else 1
                if si is not None and si.on_wait and len(si.on_wait) > cap:
                    extras = list(si.on_wait[cap:])
                    del si.on_wait[cap:]
                    for w in extras:
                        new_insts.append(
                            mybir.InstNoOp(
                                name=f"wspill-{nc.next_id()}",
                                engine=inst.engine,
                                ins=[],
                                outs=[],
                                sync_info=mybir.SyncInfo(on_wait=[w], on_update=[]),
                            )
                        )
                        n += 1
                new_insts.append(inst)
            bb.instructions[:] = new_insts
    return n


tile.TileContext._drain_and_barrier = _patched_drain_and_barrier

bass_utils.upload_artifacts = lambda tmpdir: tmpdir  # no S3 in container


def _register_ntff_hook():
    """Best-effort: enables trace=True runs (used by test harness only)."""
    try:
        try:
            from antenv.axon_hooks import set_axon_ntff_profile_hook
        except ImportError:
            # this container's antenv lacks axon_hooks — synthesize it
            import types

            import antenv

            mod = types.ModuleType("antenv.axon_hooks")
            _hook = [None]
            mod.set_axon_ntff_profile_hook = lambda h: _hook.__setitem__(0, h)
            mod.get_axon_ntff_profile_hook = lambda: _hook[0]
            sys.modules["antenv.axon_hooks"] = mod
            antenv.axon_hooks = mod
            set_axon_ntff_profile_hook = mod.set_axon_ntff_profile_hook
        sys.path.insert(0, "/root/.axon_site")
        from trn_agent_boot.trn_boot import _ntff_profile_via_ctypes

        set_axon_ntff_profile_hook(
            _ntff_profile_via_ctypes("/opt/axon/libaxon_pjrt.so")
        )
    except Exception:
        pass


# ---------------------------------------------------------------------------
# device program (identical on all 8 cores; per-core data differs)
# ---------------------------------------------------------------------------

def _build_program():
    nc = bass.Bass()

    xT = nc.declare_dram_parameter("xT", [DIM, S], BF16, isOutput=False)
    yT = nc.declare_dram_parameter("yT", [DIM, S], BF16, isOutput=False)
    wq = nc.declare_dram_parameter("wq", [DIM, FS], BF16, isOutput=False)
    wkx = nc.declare_dram_parameter("wkx", [DIM, FS], BF16, isOutput=False)
    wky = nc.declare_dram_parameter("wky", [DIM, FS], BF16, isOutput=False)
    wvx = nc.declare_dram_parameter("wvx", [DIM, FS], BF16, isOutput=False)
    wvy = nc.declare_dram_parameter("wvy", [DIM, FS], BF16, isOutput=False)
    wo = nc.declare_dram_parameter("wo", [FS, DIM], BF16, isOutput=False)
    bq = nc.declare_dram_parameter("bq", [128, 4], FP32, isOutput=False)
    bkx = nc.declare_dram_parameter("bkx", [128, 4], FP32, isOutput=False)
    bky = nc.declare_dram_parameter("bky", [128, 4], FP32, isOutput=False)
    bvx_bc = nc.declare_dram_parameter("bvx_bc", [1, FS], FP32, isOutput=False)
    bvy_bc = nc.declare_dram_parameter("bvy_bc", [1, FS], FP32, isOutput=False)
    outT = nc.declare_dram_parameter("outT", [DIM, S], FP16, isOutput=True)

    EXP = mybir.ActivationFunctionType.Exp

    with tile.TileContext(nc) as tc:
        import contextlib

        with contextlib.ExitStack() as ctx:
            big = ctx.enter_context(tc.tile_pool(name="big", bufs=24))
            wpool = ctx.enter_context(tc.tile_pool(name="wpool", bufs=40))
            qkv = ctx.enter_context(tc.tile_pool(name="qkv", bufs=12))
            vpool = ctx.enter_context(tc.tile_pool(name="vpool", bufs=16))
            ppool = ctx.enter_context(tc.tile_pool(name="ppool", bufs=6))
            opool = ctx.enter_context(tc.tile_pool(name="opool", bufs=4))
            spool = ctx.enter_context(tc.tile_pool(name="spool", bufs=4))
            cpool = ctx.enter_context(tc.tile_pool(name="cpool", bufs=1))
            dpool = ctx.enter_context(tc.tile_pool(name="dpool", bufs=8, space="DRAM"))
            mm_ps = ctx.enter_context(tc.tile_pool(name="mm_ps", bufs=3, space="PSUM"))
            ot_ps = ctx.enter_context(tc.tile_pool(name="ot_ps", bufs=2, space="PSUM"))

            # ---- constants ----
            ones_f32 = cpool.tile([128, 64], FP32, tag="ones_f32")
            nc.vector.memset(ones_f32[:, :], 1.0)
            bq_sb = cpool.tile([128, 4], FP32, tag="bq")
            bkx_sb = cpool.tile([128, 4], FP32, tag="bkx")
            bky_sb = cpool.tile([128, 4], FP32, tag="bky")
            bvx_sb = cpool.tile([128, FS], FP32, tag="bvx")
            bvy_sb = cpool.tile([128, FS], FP32, tag="bvy")

            def _bcast_ap(h):
                return bass.AP(
                    tensor=h[:, :].tensor, offset=h[:, :].offset,
                    ap=[[0, 128]] + [list(a) for a in h[:, :].ap[1:]],
                )

            # ---- tile allocations for activations + weights ----
            xt = [big.tile([128, S], BF16, tag="big", name=f"xt{i}") for i in range(8)]
            yt = [big.tile([128, S], BF16, tag="big", name=f"yt{i}") for i in range(8)]
            wo_sb = [big.tile([128, S], BF16, tag="big", name=f"wo{i}") for i in range(4)]
            wvx_sb = [wpool.tile([128, FS], BF16, tag="w", name=f"wvx{i}") for i in range(8)]
            wvy_sb = [wpool.tile([128, FS], BF16, tag="w", name=f"wvy{i}") for i in range(8)]
            qk_w = [[wpool.tile([128, FS], BF16, tag="w", name=f"wp{pi}_{ct}")
                     for ct in range(8)] for pi in range(3)]  # Q, Kx, Ky

            # ---- DMA issues: whole-row transfers (2KB lines — splitting
            # them tanks per-ring DMA efficiency), priority-ordered and
            # spread across the 3 queues that can issue DMAs.
            # sync:   wvx+xt interleaved (V-x critical path), then wq
            # scalar: yt+wvy (V-y phase, ~17us in), then wkx
            # gpsimd: biases, wky, wo
            for ct in range(8):
                nc.sync.dma_start(out=wvx_sb[ct], in_=wvx[ct * 128:(ct + 1) * 128, :])
                nc.sync.dma_start(out=xt[ct], in_=xT[ct * 128:(ct + 1) * 128, :])
            for ct in range(8):
                nc.sync.dma_start(out=qk_w[0][ct], in_=wq[ct * 128:(ct + 1) * 128, :])

            for ct in range(8):
                nc.scalar.dma_start(out=wvy_sb[ct], in_=wvy[ct * 128:(ct + 1) * 128, :])
                nc.scalar.dma_start(out=yt[ct], in_=yT[ct * 128:(ct + 1) * 128, :])
            for ct in range(8):
                nc.scalar.dma_start(out=qk_w[1][ct], in_=wkx[ct * 128:(ct + 1) * 128, :])

            nc.gpsimd.dma_start(out=bq_sb, in_=bq[:, :])
            nc.gpsimd.dma_start(out=bkx_sb, in_=bkx[:, :])
            nc.gpsimd.dma_start(out=bky_sb, in_=bky[:, :])
            nc.gpsimd.dma_start(out=bvx_sb, in_=_bcast_ap(bvx_bc))
            nc.gpsimd.dma_start(out=bvy_sb, in_=_bcast_ap(bvy_bc))
            for ct in range(8):
                nc.gpsimd.dma_start(out=qk_w[2][ct], in_=wky[ct * 128:(ct + 1) * 128, :])
            for ft in range(4):
                nc.gpsimd.dma_start(out=wo_sb[ft], in_=wo[ft * 128:(ft + 1) * 128, :])

            # ---- V projections (natural domain, bias + ones column) ----
            V = [vpool.tile([128, HG, DH + 1], BF16, tag="v", name=f"V{i}") for i in range(16)]
            for src_is_y in (False, True):
                bias_sb = bvy_sb if src_is_y else bvx_sb
                base = 8 if src_is_y else 0
                w_sb = wvy_sb if src_is_y else wvx_sb
                act = yt if src_is_y else xt
                for sg in range(4):  # two s-tiles per psum group
                    ps = mm_ps.tile([128, 1024], FP32, tag="mm", name="vps")
                    for ct in range(8):
                        for half in range(2):
                            st = 2 * sg + half
                            nc.tensor.matmul(
                                ps[:, half * 512:(half + 1) * 512],
                                act[ct][:, st * 128:(st + 1) * 128],
                                w_sb[ct][:, :],
                                start=(ct == 0),
                                stop=(ct == 7),
                            )
                    for half in range(2):
                        st = 2 * sg + half
                        vt = V[base + st]
                        nc.vector.tensor_add(
                            out=vt[:, :, 0:DH],
                            in0=ps[:, half * 512:(half + 1) * 512].rearrange(
                                "p (h d) -> p h d", h=HG),
                            in1=bias_sb[:, :].rearrange("p (h d) -> p h d", h=HG),
                        )
                        nc.vector.tensor_copy(
                            out=vt[:, :, DH:DH + 1],
                            in_=ones_f32[:, 0:HG].rearrange("p (h o) -> p h o", o=1),
                        )

            # ---- Q/K projections (transposed domain [feat, seq]) ----
            QT = [qkv.tile([128, S], BF16, tag="qkv", name=f"QT{i}") for i in range(4)]
            KxT = [qkv.tile([128, S], BF16, tag="qkv", name=f"KxT{i}") for i in range(4)]
            KyT = [qkv.tile([128, S], BF16, tag="qkv", name=f"KyT{i}") for i in range(4)]

            qk_act = [xt, xt, yt]
            qk_bias = [bq_sb, bkx_sb, bky_sb]
            qk_dst = [QT, KxT, KyT]
            qk_ps = {}

            def emit_qk_half(pi, ft, half):
                key = (pi, ft)
                if key not in qk_ps:
                    qk_ps[key] = mm_ps.tile(
                        [128, 1024], FP32, tag="mm", name=f"qkps{pi}_{ft}"
                    )
                ps = qk_ps[key]
                w_sb = qk_w[pi]
                act = qk_act[pi]
                for ct in (range(0, 4) if half == 0 else range(4, 8)):
                    for h2 in range(2):
                        nc.tensor.matmul(
                            ps[:, h2 * 512:(h2 + 1) * 512],
                            w_sb[ct][:, ft * 128:(ft + 1) * 128],
                            act[ct][:, h2 * 512:(h2 + 1) * 512],
                            start=(ct == 0),
                            stop=(ct == 7),
                        )
                if half == 1:
                    nc.vector.tensor_scalar_add(
                        out=qk_dst[pi][ft][:, :],
                        in0=ps[:, :],
                        scalar1=qk_bias[pi][:, ft:ft + 1],
                    )
                    del qk_ps[key]

            # upfront: ft=0 for all projections, plus all fts of proj 2 (wky)
            for pi in range(3):
                emit_qk_half(pi, 0, 0)
                emit_qk_half(pi, 0, 1)
            for ft in range(1, 4):
                emit_qk_half(2, ft, 0)
                emit_qk_half(2, ft, 1)

            # remaining 6 groups ride the attention phase's spare PE cycles
            qk_fillers = {(t, qt): (qt, t + 1) for t in range(3) for qt in range(2)}

            # ---- attention (head pairs row-pack; both q-tiles share one
            #      psum tile so exp runs once per (kt, head)) ----
            oT = [big.tile([128, S], BF16, tag="big", name=f"oT{i}") for i in range(4)]

            def emit_finalize(t, qt, o_sb, recips):
                for hh in range(2):
                        i = hh
                        rd = recips[i]
                        bc_sb = spool.tile([DH, 512], FP32, tag="bc", name="bc_sb")
                        rd_bcast = bass.AP(
                            tensor=rd.tensor, offset=rd.offset,
                            ap=[[0, DH]] + [list(a) for a in rd.ap[1:]],
                        )
                        nc.gpsimd.dma_start(out=bc_sb[:, :], in_=rd_bcast)
                        nc.vector.tensor_mul(
                            out=oT[t][hh * 64:hh * 64 + DH, qt * 512:(qt + 1) * 512],
                            in0=o_sb[i][:, :],
                            in1=bc_sb[:, :],
                        )

            pending = None
            for t in range(4):  # heads 2t, 2t+1
                for qt in range(2):
                    o_ps = [ot_ps.tile([128, 512], FP32, tag="ot", name=f"ops{i}")
                            for i in range(2)]  # per head of the pair
                    prev = None
                    for kt in range(16):
                        KT = KxT[t] if kt < 8 else KyT[t]
                        ks = (kt % 8) * 128
                        sc = mm_ps.tile([128, 1024], FP32, tag="mm", name="sc")
                        for hh in range(2):
                            nc.tensor.matmul(
                                sc[:, hh * 512:(hh + 1) * 512],
                                KT[hh * 64:(hh + 1) * 64, ks:ks + 128],
                                QT[t][hh * 64:(hh + 1) * 64, qt * 512:(qt + 1) * 512],
                                start=True,
                                stop=True,
                            )
                        p2 = ppool.tile([128, 1024], BF16, tag="p", name="p")
                        nc.scalar.activation(out=p2[:, :], in_=sc[:, :], func=EXP)
                        if (t, qt) in qk_fillers and kt in (4, 11):
                            fpi, fft = qk_fillers[(t, qt)]
                            emit_qk_half(fpi, fft, 0 if kt == 4 else 1)
                        if prev is not None:
                            for hh in range(2):
                                nc.tensor.matmul(
                                    o_ps[hh][0:DH + 1, :],
                                    V[kt - 1][:, 2 * t + hh, :],
                                    prev[:, hh * 512:(hh + 1) * 512],
                                    start=(kt == 1),
                                    stop=False,
                                )
                        prev = p2
                    for hh in range(2):
                        nc.tensor.matmul(
                            o_ps[hh][0:DH + 1, :],
                            V[15][:, 2 * t + hh, :],
                            prev[:, hh * 512:(hh + 1) * 512],
                            start=False,
                            stop=True,
                        )
                    if pending is not None:
                        emit_finalize(*pending)
                    o_sb = []
                    s2 = spool.tile([33, 512], FP32, tag="s2", name="s2")
                    for i in range(2):
                        nc.vector.tensor_copy(
                            out=s2[32 * i:32 * i + 1, :], in_=o_ps[i][DH:DH + 1, :]
                        )
                        ob = spool.tile([DH, 512], FP32, tag="osb", name="osb")
                        nc.vector.tensor_copy(out=ob[:, :], in_=o_ps[i][0:DH, :])
                        o_sb.append(ob)
                    rf2 = spool.tile([33, 512], FP32, tag="recipf", name="rf2")
                    nc.vector.reciprocal_approx_fast(out=rf2[:, :], in_=s2[:, :])
                    recips = []
                    for i in range(2):
                        rd = dpool.tile([1, 512], FP32, name="rd")
                        nc.gpsimd.dma_start(out=rd[:, :], in_=rf2[32 * i:32 * i + 1, :])
                        recips.append(rd)
                    pending = (t, qt, o_sb, recips)
            emit_finalize(*pending)

            # ---- out-projection (transposed domain [m, s]) ----
            # 3-deep software pipeline over the 8 m-tiles; ft0-2 first, ft3
            # (gated on the last head-pair's finalize) as late as possible.
            def op_mms(ps, mt, fts):
                for ft in fts:
                    for half in range(2):
                        nc.tensor.matmul(
                            ps[:, half * 512:(half + 1) * 512],
                            wo_sb[ft][:, mt * 128:(mt + 1) * 128],
                            oT[ft][:, half * 512:(half + 1) * 512],
                            start=(ft == 0),
                            stop=(ft == 3),
                        )

            def op_finish(ps, mt):
                osb = opool.tile([128, 1024], FP16, tag="osb2", name="osb2")
                # alternate the PSUM->SBUF cast between DVE and ACT so the
                # tail isn't serialized on one engine
                if mt % 2 == 0:
                    nc.vector.tensor_copy(out=osb[:, :], in_=ps[:, :])
                else:
                    nc.scalar.copy(out=osb[:, :], in_=ps[:, :])
                nc.sync.dma_start(
                    out=outT[mt * 128:(mt + 1) * 128, :],
                    in_=osb[:, :],
                )

            op_tiles = {}
            for mt in range(3):
                op_tiles[mt] = mm_ps.tile([128, 1024], FP32, tag="mm", name=f"obs{mt}")
                op_mms(op_tiles[mt], mt, range(3))
            for mt in range(8):
                op_mms(op_tiles[mt], mt, [3])
                op_finish(op_tiles[mt], mt)
                nxt = mt + 3
                if nxt < 8:
                    op_tiles[nxt] = mm_ps.tile([128, 1024], FP32, tag="mm", name=f"obs{nxt}")
                    op_mms(op_tiles[nxt], nxt, range(3))

    # populate .instr bytes for extended-inst InstISA subclasses (the
    # custom-DVE reciprocal) — raw Bass skips this pass and the NEFF
    # compiler errors with "ISA wrong length" without it
    mybir.codegen_inst_isa_subclasses(nc)
    _spill_excess_waits(nc)
    return nc


_NC = None


def _get_program():
    global _NC
    if _NC is None:
        _NC = _build_program()
    return _NC


# ---------------------------------------------------------------------------
# host wrapper
# ---------------------------------------------------------------------------

def _prep_in_maps(x, y, W_Kx, b_Kx, W_Qx, b_Qx, W_Vx, b_Vx, W_Ky, b_Ky,
                  W_Vy, b_Vy, W_out, b_out):
    f32 = np.float32
    bf16 = ml_dtypes.bfloat16
    in_maps = []
    for c in range(NCORES):
        b = c // 2
        g = c % 2
        gs = slice(FS * g, FS * (g + 1))
        m = {
            "xT": np.ascontiguousarray(np.asarray(x[b], f32).T).astype(bf16),
            "yT": np.ascontiguousarray(np.asarray(y[b], f32).T).astype(bf16),
            "wq": np.ascontiguousarray((np.asarray(W_Qx, f32)[gs, :] / 8.0).T).astype(bf16),
            "wkx": np.ascontiguousarray(np.asarray(W_Kx, f32)[gs, :].T).astype(bf16),
            "wky": np.ascontiguousarray(np.asarray(W_Ky, f32)[gs, :].T).astype(bf16),
            "wvx": np.ascontiguousarray(np.asarray(W_Vx, f32)[gs, :].T).astype(bf16),
            "wvy": np.ascontiguousarray(np.asarray(W_Vy, f32)[gs, :].T).astype(bf16),
            "wo": np.ascontiguousarray(np.asarray(W_out, f32)[:, gs].T).astype(bf16),
            "bq": np.ascontiguousarray(
                (np.asarray(b_Qx, f32)[gs] / 8.0).reshape(4, 128).T),
            "bkx": np.ascontiguousarray(np.asarray(b_Kx, f32)[gs].reshape(4, 128).T),
            "bky": np.ascontiguousarray(np.asarray(b_Ky, f32)[gs].reshape(4, 128).T),
            "bvx_bc": np.ascontiguousarray(np.asarray(b_Vx, f32)[gs].reshape(1, FS)),
            "bvy_bc": np.ascontiguousarray(np.asarray(b_Vy, f32)[gs].reshape(1, FS)),
        }
        in_maps.append(m)
    return in_maps


def _assemble(results, b_out):
    B = 4
    out = np.empty((B, S, DIM), np.float32)
    bo = np.asarray(b_out, np.float32)
    for b in range(B):
        acc = (results[2 * b]["outT"].astype(np.float32)
               + results[2 * b + 1]["outT"].astype(np.float32))
        out[b] = acc.T + bo
    return out


def kernel(**inputs):
    nc = _get_program()
    in_maps = _prep_in_maps(**inputs)
    last_err = None
    for _attempt in range(3):
        try:
            res = run_bass_kernel_spmd(nc, in_maps, core_ids=list(range(NCORES)))
            return _assemble(res.results, inputs["b_out"])
        except Exception as e:  # transient NRT_EXEC_UNIT_UNRECOVERABLE after fresh compile
            last_err = e
            import time as _time
            _time.sleep(2.0)
    raise last_err


def kernel_traced(trace_cores=None, **inputs):
    """Same as kernel() but returns (out, BassKernelResults) with NTFF trace."""
    _register_ntff_hook()
    nc = _get_program()
    in_maps = _prep_in_maps(**inputs)
    res = run_bass_kernel_spmd(
        nc, in_maps, core_ids=list(range(NCORES)), trace=True,
        trace_cores=trace_cores or [0],
    )
    return _assemble(res.results, inputs["b_out"]), res


# revision 20
# speedup vs baseline: 1.1244x; 1.0556x over previous
"""MultiHeadCrossAttention kernel for 8 Trainium2 NeuronCores.

Problem (hardcoded): B=4, Sx=Sy=1024, DIM=1024, H=16, Dh=64, fp32.
  Q = x@W_Qx.T+b_Qx ; K = cat(x@W_Kx.T+b_Kx, y@W_Ky.T+b_Ky) per head
  V = cat(x@W_Vx.T+b_Vx, y@W_Vy.T+b_Vy) ; out = softmax(QK^T/8)V @ W_out.T + b_out

Sharding: core c -> (batch b = c//2, head-group g = c%2 of 8 heads).
Each core computes its batch's attention for its 8 heads plus the partial
out-projection over its 512 features; host sums the two partials per batch
and adds b_out (the "all-reduce after to_out", done in the gather).

Device layout choices (all matmuls natural, zero on-device transposes):
 - activations pre-transposed on host: xT/yT [dim, seq]
 - Q/K projections in transposed domain [feat, seq]  (bias = per-partition)
 - V in natural domain [seq, feat] with host-broadcast bias, plus a ones
   column per head -> AV matmul row 64 yields the softmax denominator
 - scoresT [k, q] via lhsT=KT (d=64 contraction; head pairs row-pack the PE)
 - exp on ACT only (no max subtraction: |scores| <~ 3), normalize via
   PE-broadcast reciprocal, out-projection in transposed domain [m, s]

v2 scheduling: DMA loads split into priority slices and issued across 4
engine queues (first V matmul ~4us instead of 13); reciprocal via the
fast approx custom-DVE op; out-projection fully 3-deep pipelined right
after the last AV matmul; fp16 partial outputs (halves the output DMA).
"""

import os
import sys

os.environ.setdefault("MYCRO_LOCAL_CACHE", "1")
if "/opt/trn_rl_repo" not in sys.path:
    sys.path.insert(0, "/opt/trn_rl_repo")

import ml_dtypes
import numpy as np

import concourse.bass as bass
import concourse.mybir as mybir
import concourse.tile as tile
from concourse import bass_utils
from concourse.bass_utils import run_bass_kernel_spmd

FP32 = mybir.dt.float32
FP16 = mybir.dt.float16
BF16 = mybir.dt.bfloat16

DIM = 1024
H = 16          # total heads
HG = 8          # heads per core (head-group)
DH = 64
S = 1024        # Sx = Sy
FS = 512        # feature slice per core (HG * DH)
NCORES = 8

# ---------------------------------------------------------------------------
# harness patches (this snapshot's Tile emits >1 wait per instruction in a
# few places; HW instructions hold one wait)
# ---------------------------------------------------------------------------

def _patched_drain_and_barrier(self, tick_clock, wait_clock):
    from bass_rust import ScopedClock

    nc = self.nc
    drain_inst = nc.sync.drain()
    wait_clock.add_sem_waits(
        drain_inst.ins, ScopedClock({None: tick_clock.global_clock})
    )
    si = drain_inst.ins.sync_info
    waits = list(si.on_wait)
    if len(waits) > 1:
        del si.on_wait[1:]
        for w in waits[1:]:
            nop = nc.sync.nop(nofuse=True, hint="drain_wait_spill")
            if nop.ins.sync_info is None:
                nop.ins.sync_info = mybir.SyncInfo(on_wait=[], on_update=[])
            nop.ins.sync_info.on_wait.append(w)

    nc.all_engine_barrier()
    assert self.sems is not None
    popped = nc._tile_sem_poison_stack.pop()
    assert popped is self._sem_poison
    nc.clear_and_free_semaphores(list(self.sems.allocated().values()))
    nc.all_engine_barrier()


def _spill_excess_waits(nc):
    n = 0
    for fn in nc.m.functions:
        for bb in fn.blocks:
            new_insts = []
            for inst in bb.instructions:
                si = getattr(inst, "sync_info", None)
                cap = 2 if isinstance(inst, mybir.InstEventSemaphore) else 1
                if si is not None and si.on_wait and len(si.on_wait) > cap:
                    extras = list(si.on_wait[cap:])
                    del si.on_wait[cap:]
                    for w in extras:
                        new_insts.append(
                            mybir.InstNoOp(
                                name=f"wspill-{nc.next_id()}",
                                engine=inst.engine,
                                ins=[],
                                outs=[],
                                sync_info=mybir.SyncInfo(on_wait=[w], on_update=[]),
                            )
                        )
                        n += 1
                new_insts.append(inst)
            bb.instructions[:] = new_insts
    return n


tile.TileContext._drain_and_barrier = _patched_drain_and_barrier

bass_utils.upload_artifacts = lambda tmpdir: tmpdir  # no S3 in container


def _register_ntff_hook():
    """Best-effort: enables trace=True runs (used by test harness only)."""
    try:
        try:
            from antenv.axon_hooks import set_axon_ntff_profile_hook
        except ImportError:
            # this container's antenv lacks axon_hooks — synthesize it
            import types

            import antenv

            mod = types.ModuleType("antenv.axon_hooks")
            _hook = [None]
            mod.set_axon_ntff_profile_hook = lambda h: _hook.__setitem__(0, h)
            mod.get_axon_ntff_profile_hook = lambda: _hook[0]
            sys.modules["antenv.axon_hooks"] = mod
            antenv.axon_hooks = mod
            set_axon_ntff_profile_hook = mod.set_axon_ntff_profile_hook
        sys.path.insert(0, "/root/.axon_site")
        from trn_agent_boot.trn_boot import _ntff_profile_via_ctypes

        set_axon_ntff_profile_hook(
            _ntff_profile_via_ctypes("/opt/axon/libaxon_pjrt.so")
        )
    except Exception:
        pass


# ---------------------------------------------------------------------------
# device program (identical on all 8 cores; per-core data differs)
# ---------------------------------------------------------------------------

def _build_program():
    nc = bass.Bass()

    xT = nc.declare_dram_parameter("xT", [DIM, S], BF16, isOutput=False)
    yT = nc.declare_dram_parameter("yT", [DIM, S], BF16, isOutput=False)
    wq = nc.declare_dram_parameter("wq", [DIM, FS], BF16, isOutput=False)
    wkx = nc.declare_dram_parameter("wkx", [DIM, FS], BF16, isOutput=False)
    wky = nc.declare_dram_parameter("wky", [DIM, FS], BF16, isOutput=False)
    wvx = nc.declare_dram_parameter("wvx", [DIM, FS], BF16, isOutput=False)
    wvy = nc.declare_dram_parameter("wvy", [DIM, FS], BF16, isOutput=False)
    wo = nc.declare_dram_parameter("wo", [FS, DIM], BF16, isOutput=False)
    bq = nc.declare_dram_parameter("bq", [128, 4], FP32, isOutput=False)
    bkx = nc.declare_dram_parameter("bkx", [128, 4], FP32, isOutput=False)
    bky = nc.declare_dram_parameter("bky", [128, 4], FP32, isOutput=False)
    bvx_bc = nc.declare_dram_parameter("bvx_bc", [1, FS], FP32, isOutput=False)
    bvy_bc = nc.declare_dram_parameter("bvy_bc", [1, FS], FP32, isOutput=False)
    outT = nc.declare_dram_parameter("outT", [DIM, S], FP16, isOutput=True)

    EXP = mybir.ActivationFunctionType.Exp

    with tile.TileContext(nc) as tc:
        import contextlib

        with contextlib.ExitStack() as ctx:
            big = ctx.enter_context(tc.tile_pool(name="big", bufs=24))
            wpool = ctx.enter_context(tc.tile_pool(name="wpool", bufs=40))
            qkv = ctx.enter_context(tc.tile_pool(name="qkv", bufs=12))
            vpool = ctx.enter_context(tc.tile_pool(name="vpool", bufs=16))
            ppool = ctx.enter_context(tc.tile_pool(name="ppool", bufs=6))
            opool = ctx.enter_context(tc.tile_pool(name="opool", bufs=4))
            spool = ctx.enter_context(tc.tile_pool(name="spool", bufs=4))
            cpool = ctx.enter_context(tc.tile_pool(name="cpool", bufs=1))
            dpool = ctx.enter_context(tc.tile_pool(name="dpool", bufs=8, space="DRAM"))
            mm_ps = ctx.enter_context(tc.tile_pool(name="mm_ps", bufs=3, space="PSUM"))
            ot_ps = ctx.enter_context(tc.tile_pool(name="ot_ps", bufs=2, space="PSUM"))

            # ---- constants ----
            ones_f32 = cpool.tile([128, 64], FP32, tag="ones_f32")
            nc.vector.memset(ones_f32[:, :], 1.0)
            bq_sb = cpool.tile([128, 4], FP32, tag="bq")
            bkx_sb = cpool.tile([128, 4], FP32, tag="bkx")
            bky_sb = cpool.tile([128, 4], FP32, tag="bky")
            bvx_sb = cpool.tile([128, FS], FP32, tag="bvx")
            bvy_sb = cpool.tile([128, FS], FP32, tag="bvy")

            def _bcast_ap(h):
                return bass.AP(
                    tensor=h[:, :].tensor, offset=h[:, :].offset,
                    ap=[[0, 128]] + [list(a) for a in h[:, :].ap[1:]],
                )

            # ---- tile allocations for activations + weights ----
            xt = [big.tile([128, S], BF16, tag="big", name=f"xt{i}") for i in range(8)]
            yt = [big.tile([128, S], BF16, tag="big", name=f"yt{i}") for i in range(8)]
            wo_sb = [big.tile([128, S], BF16, tag="big", name=f"wo{i}") for i in range(4)]
            wvx_sb = [wpool.tile([128, FS], BF16, tag="w", name=f"wvx{i}") for i in range(8)]
            wvy_sb = [wpool.tile([128, FS], BF16, tag="w", name=f"wvy{i}") for i in range(8)]
            qk_w = [[wpool.tile([128, FS], BF16, tag="w", name=f"wp{pi}_{ct}")
                     for ct in range(8)] for pi in range(3)]  # Q, Kx, Ky

            # ---- DMA issues: single-queue priority order (HBM bandwidth is
            # shared, so the critical V-x path must not compete with later
            # loads — parallel queues made it WORSE). Baseline order, with
            # the first xt chunk split across two rings for an earlier
            # first matmul.
            nc.sync.dma_start(out=wvx_sb[0], in_=wvx[0:128, :])
            nc.sync.dma_start(out=xt[0][:, 0:512], in_=xT[0:128, 0:512])
            nc.sync.dma_start(out=xt[0][:, 512:1024], in_=xT[0:128, 512:1024])
            for ct in range(1, 8):
                nc.sync.dma_start(out=wvx_sb[ct], in_=wvx[ct * 128:(ct + 1) * 128, :])
                nc.sync.dma_start(out=xt[ct], in_=xT[ct * 128:(ct + 1) * 128, :])
            for ct in range(8):
                nc.sync.dma_start(out=wvy_sb[ct], in_=wvy[ct * 128:(ct + 1) * 128, :])
                nc.sync.dma_start(out=yt[ct], in_=yT[ct * 128:(ct + 1) * 128, :])
            for ct in range(8):
                nc.sync.dma_start(out=qk_w[0][ct], in_=wq[ct * 128:(ct + 1) * 128, :])
            for ct in range(8):
                nc.sync.dma_start(out=qk_w[1][ct], in_=wkx[ct * 128:(ct + 1) * 128, :])
            for ct in range(8):
                nc.sync.dma_start(out=qk_w[2][ct], in_=wky[ct * 128:(ct + 1) * 128, :])
            for ft in range(4):
                nc.sync.dma_start(out=wo_sb[ft], in_=wo[ft * 128:(ft + 1) * 128, :])

            nc.gpsimd.dma_start(out=bq_sb, in_=bq[:, :])
            nc.gpsimd.dma_start(out=bkx_sb, in_=bkx[:, :])
            nc.gpsimd.dma_start(out=bky_sb, in_=bky[:, :])
            nc.gpsimd.dma_start(out=bvx_sb, in_=_bcast_ap(bvx_bc))
            nc.gpsimd.dma_start(out=bvy_sb, in_=_bcast_ap(bvy_bc))

            # ---- V projections (natural domain, bias + ones column) ----
            V = [vpool.tile([128, HG, DH + 1], BF16, tag="v", name=f"V{i}") for i in range(16)]
            for src_is_y in (False, True):
                bias_sb = bvy_sb if src_is_y else bvx_sb
                base = 8 if src_is_y else 0
                w_sb = wvy_sb if src_is_y else wvx_sb
                act = yt if src_is_y else xt
                for sg in range(4):  # two s-tiles per psum group
                    ps = mm_ps.tile([128, 1024], FP32, tag="mm", name="vps")
                    for ct in range(8):
                        for half in range(2):
                            st = 2 * sg + half
                            nc.tensor.matmul(
                                ps[:, half * 512:(half + 1) * 512],
                                act[ct][:, st * 128:(st + 1) * 128],
                                w_sb[ct][:, :],
                                start=(ct == 0),
                                stop=(ct == 7),
                            )
                    for half in range(2):
                        st = 2 * sg + half
                        vt = V[base + st]
                        nc.vector.tensor_add(
                            out=vt[:, :, 0:DH],
                            in0=ps[:, half * 512:(half + 1) * 512].rearrange(
                                "p (h d) -> p h d", h=HG),
                            in1=bias_sb[:, :].rearrange("p (h d) -> p h d", h=HG),
                        )
                        nc.vector.tensor_copy(
                            out=vt[:, :, DH:DH + 1],
                            in_=ones_f32[:, 0:HG].rearrange("p (h o) -> p h o", o=1),
                        )

            # ---- Q/K projections (transposed domain [feat, seq]) ----
            QT = [qkv.tile([128, S], BF16, tag="qkv", name=f"QT{i}") for i in range(4)]
            KxT = [qkv.tile([128, S], BF16, tag="qkv", name=f"KxT{i}") for i in range(4)]
            KyT = [qkv.tile([128, S], BF16, tag="qkv", name=f"KyT{i}") for i in range(4)]

            qk_act = [xt, xt, yt]
            qk_bias = [bq_sb, bkx_sb, bky_sb]
            qk_dst = [QT, KxT, KyT]
            qk_ps = {}

            def emit_qk_half(pi, ft, half):
                key = (pi, ft)
                if key not in qk_ps:
                    qk_ps[key] = mm_ps.tile(
                        [128, 1024], FP32, tag="mm", name=f"qkps{pi}_{ft}"
                    )
                ps = qk_ps[key]
                w_sb = qk_w[pi]
                act = qk_act[pi]
                for ct in (range(0, 4) if half == 0 else range(4, 8)):
                    for h2 in range(2):
                        nc.tensor.matmul(
                            ps[:, h2 * 512:(h2 + 1) * 512],
                            w_sb[ct][:, ft * 128:(ft + 1) * 128],
                            act[ct][:, h2 * 512:(h2 + 1) * 512],
                            start=(ct == 0),
                            stop=(ct == 7),
                        )
                if half == 1:
                    nc.vector.tensor_scalar_add(
                        out=qk_dst[pi][ft][:, :],
                        in0=ps[:, :],
                        scalar1=qk_bias[pi][:, ft:ft + 1],
                    )
                    del qk_ps[key]

            # upfront: ft=0 for all projections, plus all fts of proj 2 (wky)
            for pi in range(3):
                emit_qk_half(pi, 0, 0)
                emit_qk_half(pi, 0, 1)
            for ft in range(1, 4):
                emit_qk_half(2, ft, 0)
                emit_qk_half(2, ft, 1)

            # remaining 6 groups ride the attention phase's spare PE cycles
            qk_fillers = {(t, qt): (qt, t + 1) for t in range(3) for qt in range(2)}

            # ---- attention (head pairs row-pack; both q-tiles share one
            #      psum tile so exp runs once per (kt, head)) ----
            oT = [big.tile([128, S], BF16, tag="big", name=f"oT{i}") for i in range(4)]

            def emit_finalize(t, qt, o_sb, recips):
                for hh in range(2):
                        i = hh
                        rd = recips[i]
                        bc_sb = spool.tile([DH, 512], FP32, tag="bc", name="bc_sb")
                        rd_bcast = bass.AP(
                            tensor=rd.tensor, offset=rd.offset,
                            ap=[[0, DH]] + [list(a) for a in rd.ap[1:]],
                        )
                        nc.gpsimd.dma_start(out=bc_sb[:, :], in_=rd_bcast)
                        nc.vector.tensor_mul(
                            out=oT[t][hh * 64:hh * 64 + DH, qt * 512:(qt + 1) * 512],
                            in0=o_sb[i][:, :],
                            in1=bc_sb[:, :],
                        )

            def emit_finalize_fast(t, qt, o_sb, rfs):
                # last-group variant: broadcast the reciprocal via a K=1
                # PE matmul into a just-freed PSUM bank instead of the DRAM
                # bounce — ~2us chain instead of ~7us, and the PE never
                # idles long enough for HAM to re-throttle.
                for hh in range(2):
                    bc_ps = ot_ps.tile([128, 512], FP32, tag="ot", name="bc_ps")
                    nc.tensor.matmul(
                        bc_ps[0:DH, :],
                        ones_f32[0:1, 0:DH],
                        rfs[hh][0:1, :],
                        start=True,
                        stop=True,
                    )
                    nc.vector.tensor_mul(
                        out=oT[t][hh * 64:hh * 64 + DH, qt * 512:(qt + 1) * 512],
                        in0=o_sb[hh][:, :],
                        in1=bc_ps[0:DH, :],
                    )

            pending = None
            for t in range(4):  # heads 2t, 2t+1
                for qt in range(2):
                    o_ps = [ot_ps.tile([128, 512], FP32, tag="ot", name=f"ops{i}")
                            for i in range(2)]  # per head of the pair
                    prev = None
                    for kt in range(16):
                        KT = KxT[t] if kt < 8 else KyT[t]
                        ks = (kt % 8) * 128
                        sc = mm_ps.tile([128, 1024], FP32, tag="mm", name="sc")
                        for hh in range(2):
                            nc.tensor.matmul(
                                sc[:, hh * 512:(hh + 1) * 512],
                                KT[hh * 64:(hh + 1) * 64, ks:ks + 128],
                                QT[t][hh * 64:(hh + 1) * 64, qt * 512:(qt + 1) * 512],
                                start=True,
                                stop=True,
                            )
                        p2 = ppool.tile([128, 1024], BF16, tag="p", name="p")
                        nc.scalar.activation(out=p2[:, :], in_=sc[:, :], func=EXP)
                        if (t, qt) in qk_fillers and kt in (4, 11):
                            fpi, fft = qk_fillers[(t, qt)]
                            emit_qk_half(fpi, fft, 0 if kt == 4 else 1)
                        if prev is not None:
                            for hh in range(2):
                                nc.tensor.matmul(
                                    o_ps[hh][0:DH + 1, :],
                                    V[kt - 1][:, 2 * t + hh, :],
                                    prev[:, hh * 512:(hh + 1) * 512],
                                    start=(kt == 1),
                                    stop=False,
                                )
                        prev = p2
                    for hh in range(2):
                        nc.tensor.matmul(
                            o_ps[hh][0:DH + 1, :],
                            V[15][:, 2 * t + hh, :],
                            prev[:, hh * 512:(hh + 1) * 512],
                            start=False,
                            stop=True,
                        )
                    if pending is not None:
                        emit_finalize(*pending)
                        pending = None
                    is_last = (t == 3 and qt == 1)
                    o_sb = []
                    rfs = []
                    recips = []
                    # den copies + reciprocals first so the (last-group) PE
                    # broadcast can start as early as possible
                    for i in range(2):
                        s2h = spool.tile([1, 512], FP32, tag="s2", name="s2h")
                        nc.vector.tensor_copy(out=s2h[:, :], in_=o_ps[i][DH:DH + 1, :])
                        rfh = spool.tile([1, 512], FP32, tag="recipf", name="rfh")
                        nc.vector.reciprocal_approx_fast(out=rfh[:, :], in_=s2h[:, :])
                        rfs.append(rfh)
                    for i in range(2):
                        ob = spool.tile([DH, 512], FP32, tag="osb", name="osb")
                        nc.vector.tensor_copy(out=ob[:, :], in_=o_ps[i][0:DH, :])
                        o_sb.append(ob)
                        if not is_last:
                            rd = dpool.tile([1, 512], FP32, name="rd")
                            nc.gpsimd.dma_start(out=rd[:, :], in_=rfs[i][0:1, :])
                            recips.append(rd)
                    if is_last:
                        last_fast = (t, qt, o_sb, rfs)
                    else:
                        pending = (t, qt, o_sb, recips)
            if pending is not None:
                emit_finalize(*pending)
            emit_finalize_fast(*last_fast)

            # ---- out-projection (transposed domain [m, s]) ----
            # 3-deep software pipeline over the 8 m-tiles; ft0-2 first, ft3
            # (gated on the last head-pair's finalize) as late as possible.
            def op_mms(ps, mt, fts):
                for ft in fts:
                    for half in range(2):
                        nc.tensor.matmul(
                            ps[:, half * 512:(half + 1) * 512],
                            wo_sb[ft][:, mt * 128:(mt + 1) * 128],
                            oT[ft][:, half * 512:(half + 1) * 512],
                            start=(ft == 0),
                            stop=(ft == 3),
                        )

            def op_finish(ps, mt):
                osb = opool.tile([128, 1024], FP16, tag="osb2", name="osb2")
                # alternate the PSUM->SBUF cast between DVE and ACT so the
                # tail isn't serialized on one engine; split the store over
                # two DMA rings so the last transfer clears quickly
                if mt % 2 == 0:
                    nc.vector.tensor_copy(out=osb[:, :], in_=ps[:, :])
                    eng = nc.sync
                else:
                    nc.scalar.copy(out=osb[:, :], in_=ps[:, :])
                    eng = nc.scalar
                eng.dma_start(out=outT[mt * 128:(mt + 1) * 128, 0:512],
                              in_=osb[:, 0:512])
                eng.dma_start(out=outT[mt * 128:(mt + 1) * 128, 512:1024],
                              in_=osb[:, 512:1024])

            op_tiles = {}
            for mt in range(3):
                op_tiles[mt] = mm_ps.tile([128, 1024], FP32, tag="mm", name=f"obs{mt}")
                op_mms(op_tiles[mt], mt, range(3))
            for mt in range(8):
                op_mms(op_tiles[mt], mt, [3])
                op_finish(op_tiles[mt], mt)
                nxt = mt + 3
                if nxt < 8:
                    op_tiles[nxt] = mm_ps.tile([128, 1024], FP32, tag="mm", name=f"obs{nxt}")
                    op_mms(op_tiles[nxt], nxt, range(3))

    # populate .instr bytes for extended-inst InstISA subclasses (the
    # custom-DVE reciprocal) — raw Bass skips this pass and the NEFF
    # compiler errors with "ISA wrong length" without it
    mybir.codegen_inst_isa_subclasses(nc)
    _spill_excess_waits(nc)
    return nc


_NC = None


def _get_program():
    global _NC
    if _NC is None:
        _NC = _build_program()
    return _NC


# ---------------------------------------------------------------------------
# host wrapper
# ---------------------------------------------------------------------------

def _prep_in_maps(x, y, W_Kx, b_Kx, W_Qx, b_Qx, W_Vx, b_Vx, W_Ky, b_Ky,
                  W_Vy, b_Vy, W_out, b_out):
    f32 = np.float32
    bf16 = ml_dtypes.bfloat16
    in_maps = []
    for c in range(NCORES):
        b = c // 2
        g = c % 2
        gs = slice(FS * g, FS * (g + 1))
        m = {
            "xT": np.ascontiguousarray(np.asarray(x[b], f32).T).astype(bf16),
            "yT": np.ascontiguousarray(np.asarray(y[b], f32).T).astype(bf16),
            "wq": np.ascontiguousarray((np.asarray(W_Qx, f32)[gs, :] / 8.0).T).astype(bf16),
            "wkx": np.ascontiguousarray(np.asarray(W_Kx, f32)[gs, :].T).astype(bf16),
            "wky": np.ascontiguousarray(np.asarray(W_Ky, f32)[gs, :].T).astype(bf16),
            "wvx": np.ascontiguousarray(np.asarray(W_Vx, f32)[gs, :].T).astype(bf16),
            "wvy": np.ascontiguousarray(np.asarray(W_Vy, f32)[gs, :].T).astype(bf16),
            "wo": np.ascontiguousarray(np.asarray(W_out, f32)[:, gs].T).astype(bf16),
            "bq": np.ascontiguousarray(
                (np.asarray(b_Qx, f32)[gs] / 8.0).reshape(4, 128).T),
            "bkx": np.ascontiguousarray(np.asarray(b_Kx, f32)[gs].reshape(4, 128).T),
            "bky": np.ascontiguousarray(np.asarray(b_Ky, f32)[gs].reshape(4, 128).T),
            "bvx_bc": np.ascontiguousarray(np.asarray(b_Vx, f32)[gs].reshape(1, FS)),
            "bvy_bc": np.ascontiguousarray(np.asarray(b_Vy, f32)[gs].reshape(1, FS)),
        }
        in_maps.append(m)
    return in_maps


def _assemble(results, b_out):
    B = 4
    out = np.empty((B, S, DIM), np.float32)
    bo = np.asarray(b_out, np.float32)
    for b in range(B):
        acc = (results[2 * b]["outT"].astype(np.float32)
               + results[2 * b + 1]["outT"].astype(np.float32))
        out[b] = acc.T + bo
    return out


def kernel(**inputs):
    nc = _get_program()
    in_maps = _prep_in_maps(**inputs)
    last_err = None
    for _attempt in range(3):
        try:
            res = run_bass_kernel_spmd(nc, in_maps, core_ids=list(range(NCORES)))
            return _assemble(res.results, inputs["b_out"])
        except Exception as e:  # transient NRT_EXEC_UNIT_UNRECOVERABLE after fresh compile
            last_err = e
            import time as _time
            _time.sleep(2.0)
    raise last_err


def kernel_traced(trace_cores=None, **inputs):
    """Same as kernel() but returns (out, BassKernelResults) with NTFF trace."""
    _register_ntff_hook()
    nc = _get_program()
    in_maps = _prep_in_maps(**inputs)
    res = run_bass_kernel_spmd(
        nc, in_maps, core_ids=list(range(NCORES)), trace=True,
        trace_cores=trace_cores or [0],
    )
    return _assemble(res.results, inputs["b_out"]), res


# revision 22
# speedup vs baseline: 1.1326x; 1.0073x over previous
"""MultiHeadCrossAttention kernel for 8 Trainium2 NeuronCores.

Problem (hardcoded): B=4, Sx=Sy=1024, DIM=1024, H=16, Dh=64, fp32.
  Q = x@W_Qx.T+b_Qx ; K = cat(x@W_Kx.T+b_Kx, y@W_Ky.T+b_Ky) per head
  V = cat(x@W_Vx.T+b_Vx, y@W_Vy.T+b_Vy) ; out = softmax(QK^T/8)V @ W_out.T + b_out

Sharding: core c -> (batch b = c//2, head-group g = c%2 of 8 heads).
Each core computes its batch's attention for its 8 heads plus the partial
out-projection over its 512 features; host sums the two partials per batch
and adds b_out (the "all-reduce after to_out", done in the gather).

Device layout choices (all matmuls natural, zero on-device transposes):
 - activations pre-transposed on host: xT/yT [dim, seq]
 - Q/K projections in transposed domain [feat, seq]  (bias = per-partition)
 - V in natural domain [seq, feat] with host-broadcast bias, plus a ones
   column per head -> AV matmul row 64 yields the softmax denominator
 - scoresT [k, q] via lhsT=KT (d=64 contraction; head pairs row-pack the PE)
 - exp on ACT only (no max subtraction: |scores| <~ 3), normalize via
   PE-broadcast reciprocal, out-projection in transposed domain [m, s]

v2 scheduling: DMA loads split into priority slices and issued across 4
engine queues (first V matmul ~4us instead of 13); reciprocal via the
fast approx custom-DVE op; out-projection fully 3-deep pipelined right
after the last AV matmul; fp16 partial outputs (halves the output DMA).
"""

import os
import sys

os.environ.setdefault("MYCRO_LOCAL_CACHE", "1")
if "/opt/trn_rl_repo" not in sys.path:
    sys.path.insert(0, "/opt/trn_rl_repo")

import ml_dtypes
import numpy as np

import concourse.bass as bass
import concourse.mybir as mybir
import concourse.tile as tile
from concourse import bass_utils
from concourse.bass_utils import run_bass_kernel_spmd

FP32 = mybir.dt.float32
FP16 = mybir.dt.float16
BF16 = mybir.dt.bfloat16

DIM = 1024
H = 16          # total heads
HG = 8          # heads per core (head-group)
DH = 64
S = 1024        # Sx = Sy
FS = 512        # feature slice per core (HG * DH)
NCORES = 8

# ---------------------------------------------------------------------------
# harness patches (this snapshot's Tile emits >1 wait per instruction in a
# few places; HW instructions hold one wait)
# ---------------------------------------------------------------------------

def _patched_drain_and_barrier(self, tick_clock, wait_clock):
    from bass_rust import ScopedClock

    nc = self.nc
    drain_inst = nc.sync.drain()
    wait_clock.add_sem_waits(
        drain_inst.ins, ScopedClock({None: tick_clock.global_clock})
    )
    si = drain_inst.ins.sync_info
    waits = list(si.on_wait)
    if len(waits) > 1:
        del si.on_wait[1:]
        for w in waits[1:]:
            nop = nc.sync.nop(nofuse=True, hint="drain_wait_spill")
            if nop.ins.sync_info is None:
                nop.ins.sync_info = mybir.SyncInfo(on_wait=[], on_update=[])
            nop.ins.sync_info.on_wait.append(w)

    nc.all_engine_barrier()
    assert self.sems is not None
    popped = nc._tile_sem_poison_stack.pop()
    assert popped is self._sem_poison
    nc.clear_and_free_semaphores(list(self.sems.allocated().values()))
    nc.all_engine_barrier()


def _spill_excess_waits(nc):
    n = 0
    for fn in nc.m.functions:
        for bb in fn.blocks:
            new_insts = []
            for inst in bb.instructions:
                si = getattr(inst, "sync_info", None)
                cap = 2 if isinstance(inst, mybir.InstEventSemaphore) else 1
                if si is not None and si.on_wait and len(si.on_wait) > cap:
                    extras = list(si.on_wait[cap:])
                    del si.on_wait[cap:]
                    for w in extras:
                        new_insts.append(
                            mybir.InstNoOp(
                                name=f"wspill-{nc.next_id()}",
                                engine=inst.engine,
                                ins=[],
                                outs=[],
                                sync_info=mybir.SyncInfo(on_wait=[w], on_update=[]),
                            )
                        )
                        n += 1
                new_insts.append(inst)
            bb.instructions[:] = new_insts
    return n


tile.TileContext._drain_and_barrier = _patched_drain_and_barrier

bass_utils.upload_artifacts = lambda tmpdir: tmpdir  # no S3 in container


def _register_ntff_hook():
    """Best-effort: enables trace=True runs (used by test harness only)."""
    try:
        try:
            from antenv.axon_hooks import set_axon_ntff_profile_hook
        except ImportError:
            # this container's antenv lacks axon_hooks — synthesize it
            import types

            import antenv

            mod = types.ModuleType("antenv.axon_hooks")
            _hook = [None]
            mod.set_axon_ntff_profile_hook = lambda h: _hook.__setitem__(0, h)
            mod.get_axon_ntff_profile_hook = lambda: _hook[0]
            sys.modules["antenv.axon_hooks"] = mod
            antenv.axon_hooks = mod
            set_axon_ntff_profile_hook = mod.set_axon_ntff_profile_hook
        sys.path.insert(0, "/root/.axon_site")
        from trn_agent_boot.trn_boot import _ntff_profile_via_ctypes

        set_axon_ntff_profile_hook(
            _ntff_profile_via_ctypes("/opt/axon/libaxon_pjrt.so")
        )
    except Exception:
        pass


# ---------------------------------------------------------------------------
# device program (identical on all 8 cores; per-core data differs)
# ---------------------------------------------------------------------------

def _build_program():
    nc = bass.Bass()

    xT = nc.declare_dram_parameter("xT", [DIM, S], BF16, isOutput=False)
    yT = nc.declare_dram_parameter("yT", [DIM, S], BF16, isOutput=False)
    wq = nc.declare_dram_parameter("wq", [DIM, FS], BF16, isOutput=False)
    wkx = nc.declare_dram_parameter("wkx", [DIM, FS], BF16, isOutput=False)
    wky = nc.declare_dram_parameter("wky", [DIM, FS], BF16, isOutput=False)
    wvx = nc.declare_dram_parameter("wvx", [DIM, FS], BF16, isOutput=False)
    wvy = nc.declare_dram_parameter("wvy", [DIM, FS], BF16, isOutput=False)
    wo = nc.declare_dram_parameter("wo", [FS, DIM], BF16, isOutput=False)
    bq = nc.declare_dram_parameter("bq", [128, 4], FP32, isOutput=False)
    bkx = nc.declare_dram_parameter("bkx", [128, 4], FP32, isOutput=False)
    bky = nc.declare_dram_parameter("bky", [128, 4], FP32, isOutput=False)
    bvx_bc = nc.declare_dram_parameter("bvx_bc", [1, FS], FP32, isOutput=False)
    bvy_bc = nc.declare_dram_parameter("bvy_bc", [1, FS], FP32, isOutput=False)
    outT = nc.declare_dram_parameter("outT", [DIM, S], FP16, isOutput=True)

    EXP = mybir.ActivationFunctionType.Exp

    with tile.TileContext(nc) as tc:
        import contextlib

        with contextlib.ExitStack() as ctx:
            big = ctx.enter_context(tc.tile_pool(name="big", bufs=24))
            wpool = ctx.enter_context(tc.tile_pool(name="wpool", bufs=40))
            qkv = ctx.enter_context(tc.tile_pool(name="qkv", bufs=12))
            vpool = ctx.enter_context(tc.tile_pool(name="vpool", bufs=16))
            ppool = ctx.enter_context(tc.tile_pool(name="ppool", bufs=6))
            opool = ctx.enter_context(tc.tile_pool(name="opool", bufs=4))
            spool = ctx.enter_context(tc.tile_pool(name="spool", bufs=4))
            cpool = ctx.enter_context(tc.tile_pool(name="cpool", bufs=1))
            dpool = ctx.enter_context(tc.tile_pool(name="dpool", bufs=8, space="DRAM"))
            mm_ps = ctx.enter_context(tc.tile_pool(name="mm_ps", bufs=3, space="PSUM"))
            ot_ps = ctx.enter_context(tc.tile_pool(name="ot_ps", bufs=2, space="PSUM"))

            # ---- constants ----
            ones_f32 = cpool.tile([128, 64], FP32, tag="ones_f32")
            nc.vector.memset(ones_f32[:, :], 1.0)
            bq_sb = cpool.tile([128, 4], FP32, tag="bq")
            bkx_sb = cpool.tile([128, 4], FP32, tag="bkx")
            bky_sb = cpool.tile([128, 4], FP32, tag="bky")
            bvx_sb = cpool.tile([128, FS], FP32, tag="bvx")
            bvy_sb = cpool.tile([128, FS], FP32, tag="bvy")

            def _bcast_ap(h):
                return bass.AP(
                    tensor=h[:, :].tensor, offset=h[:, :].offset,
                    ap=[[0, 128]] + [list(a) for a in h[:, :].ap[1:]],
                )

            # ---- tile allocations for activations + weights ----
            xt = [big.tile([128, S], BF16, tag="big", name=f"xt{i}") for i in range(8)]
            yt = [big.tile([128, S], BF16, tag="big", name=f"yt{i}") for i in range(8)]
            wo_sb = [big.tile([128, S], BF16, tag="big", name=f"wo{i}") for i in range(4)]
            wvx_sb = [wpool.tile([128, FS], BF16, tag="w", name=f"wvx{i}") for i in range(8)]
            wvy_sb = [wpool.tile([128, FS], BF16, tag="w", name=f"wvy{i}") for i in range(8)]
            qk_w = [[wpool.tile([128, FS], BF16, tag="w", name=f"wp{pi}_{ct}")
                     for ct in range(8)] for pi in range(3)]  # Q, Kx, Ky

            # ---- DMA issues: single-queue priority order (HBM bandwidth is
            # shared, so the critical V-x path must not compete with later
            # loads — parallel queues made it WORSE). Baseline order, with
            # the first xt chunk split across two rings for an earlier
            # first matmul.
            for ct in range(8):
                nc.sync.dma_start(out=wvx_sb[ct], in_=wvx[ct * 128:(ct + 1) * 128, :])
                nc.sync.dma_start(out=xt[ct], in_=xT[ct * 128:(ct + 1) * 128, :])
            for ct in range(8):
                nc.sync.dma_start(out=wvy_sb[ct], in_=wvy[ct * 128:(ct + 1) * 128, :])
                nc.sync.dma_start(out=yt[ct], in_=yT[ct * 128:(ct + 1) * 128, :])
            for ct in range(8):
                nc.sync.dma_start(out=qk_w[0][ct], in_=wq[ct * 128:(ct + 1) * 128, :])
            for ct in range(8):
                nc.sync.dma_start(out=qk_w[1][ct], in_=wkx[ct * 128:(ct + 1) * 128, :])
            for ct in range(8):
                nc.sync.dma_start(out=qk_w[2][ct], in_=wky[ct * 128:(ct + 1) * 128, :])
            for ft in range(4):
                nc.sync.dma_start(out=wo_sb[ft], in_=wo[ft * 128:(ft + 1) * 128, :])

            nc.gpsimd.dma_start(out=bq_sb, in_=bq[:, :])
            nc.gpsimd.dma_start(out=bkx_sb, in_=bkx[:, :])
            nc.gpsimd.dma_start(out=bky_sb, in_=bky[:, :])
            nc.gpsimd.dma_start(out=bvx_sb, in_=_bcast_ap(bvx_bc))
            nc.gpsimd.dma_start(out=bvy_sb, in_=_bcast_ap(bvy_bc))

            # ---- V projections (natural domain, bias + ones column) ----
            V = [vpool.tile([128, HG, DH + 1], BF16, tag="v", name=f"V{i}") for i in range(16)]
            for src_is_y in (False, True):
                bias_sb = bvy_sb if src_is_y else bvx_sb
                base = 8 if src_is_y else 0
                w_sb = wvy_sb if src_is_y else wvx_sb
                act = yt if src_is_y else xt
                for sg in range(4):  # two s-tiles per psum group
                    ps = mm_ps.tile([128, 1024], FP32, tag="mm", name="vps")
                    for ct in range(8):
                        for half in range(2):
                            st = 2 * sg + half
                            nc.tensor.matmul(
                                ps[:, half * 512:(half + 1) * 512],
                                act[ct][:, st * 128:(st + 1) * 128],
                                w_sb[ct][:, :],
                                start=(ct == 0),
                                stop=(ct == 7),
                            )
                    for half in range(2):
                        st = 2 * sg + half
                        vt = V[base + st]
                        nc.vector.tensor_add(
                            out=vt[:, :, 0:DH],
                            in0=ps[:, half * 512:(half + 1) * 512].rearrange(
                                "p (h d) -> p h d", h=HG),
                            in1=bias_sb[:, :].rearrange("p (h d) -> p h d", h=HG),
                        )
                        nc.vector.tensor_copy(
                            out=vt[:, :, DH:DH + 1],
                            in_=ones_f32[:, 0:HG].rearrange("p (h o) -> p h o", o=1),
                        )

            # ---- Q/K projections (transposed domain [feat, seq]) ----
            QT = [qkv.tile([128, S], BF16, tag="qkv", name=f"QT{i}") for i in range(4)]
            KxT = [qkv.tile([128, S], BF16, tag="qkv", name=f"KxT{i}") for i in range(4)]
            KyT = [qkv.tile([128, S], BF16, tag="qkv", name=f"KyT{i}") for i in range(4)]

            qk_act = [xt, xt, yt]
            qk_bias = [bq_sb, bkx_sb, bky_sb]
            qk_dst = [QT, KxT, KyT]
            qk_ps = {}

            def emit_qk_half(pi, ft, half):
                key = (pi, ft)
                if key not in qk_ps:
                    qk_ps[key] = mm_ps.tile(
                        [128, 1024], FP32, tag="mm", name=f"qkps{pi}_{ft}"
                    )
                ps = qk_ps[key]
                w_sb = qk_w[pi]
                act = qk_act[pi]
                for ct in (range(0, 4) if half == 0 else range(4, 8)):
                    for h2 in range(2):
                        nc.tensor.matmul(
                            ps[:, h2 * 512:(h2 + 1) * 512],
                            w_sb[ct][:, ft * 128:(ft + 1) * 128],
                            act[ct][:, h2 * 512:(h2 + 1) * 512],
                            start=(ct == 0),
                            stop=(ct == 7),
                        )
                if half == 1:
                    nc.vector.tensor_scalar_add(
                        out=qk_dst[pi][ft][:, :],
                        in0=ps[:, :],
                        scalar1=qk_bias[pi][:, ft:ft + 1],
                    )
                    del qk_ps[key]

            # upfront: ft=0 for all projections, plus all fts of proj 2 (wky)
            for pi in range(3):
                emit_qk_half(pi, 0, 0)
                emit_qk_half(pi, 0, 1)
            for ft in range(1, 4):
                emit_qk_half(2, ft, 0)
                emit_qk_half(2, ft, 1)

            # remaining 6 groups ride the attention phase's spare PE cycles
            qk_fillers = {(t, qt): (qt, t + 1) for t in range(3) for qt in range(2)}

            # ---- attention (head pairs row-pack; both q-tiles share one
            #      psum tile so exp runs once per (kt, head)) ----
            oT = [big.tile([128, S], BF16, tag="big", name=f"oT{i}") for i in range(4)]

            def emit_finalize(t, qt, o_sb, recips):
                for hh in range(2):
                        i = hh
                        rd = recips[i]
                        bc_sb = spool.tile([DH, 512], FP32, tag="bc", name="bc_sb")
                        rd_bcast = bass.AP(
                            tensor=rd.tensor, offset=rd.offset,
                            ap=[[0, DH]] + [list(a) for a in rd.ap[1:]],
                        )
                        nc.gpsimd.dma_start(out=bc_sb[:, :], in_=rd_bcast)
                        nc.vector.tensor_mul(
                            out=oT[t][hh * 64:hh * 64 + DH, qt * 512:(qt + 1) * 512],
                            in0=o_sb[i][:, :],
                            in1=bc_sb[:, :],
                        )

            def emit_finalize_fast(t, qt, o_sb, rfs):
                # last-group variant: broadcast the reciprocal via a K=1
                # PE matmul into a just-freed PSUM bank instead of the DRAM
                # bounce — ~2us chain instead of ~7us, and the PE never
                # idles long enough for HAM to re-throttle.
                for hh in range(2):
                    bc_ps = ot_ps.tile([128, 512], FP32, tag="ot", name="bc_ps")
                    nc.tensor.matmul(
                        bc_ps[0:DH, :],
                        ones_f32[0:1, 0:DH],
                        rfs[hh][0:1, :],
                        start=True,
                        stop=True,
                    )
                    nc.vector.tensor_mul(
                        out=oT[t][hh * 64:hh * 64 + DH, qt * 512:(qt + 1) * 512],
                        in0=o_sb[hh][:, :],
                        in1=bc_ps[0:DH, :],
                    )

            pending = None
            for t in range(4):  # heads 2t, 2t+1
                for qt in range(2):
                    o_ps = [ot_ps.tile([128, 512], FP32, tag="ot", name=f"ops{i}")
                            for i in range(2)]  # per head of the pair
                    hist = []
                    for kt in range(16):
                        KT = KxT[t] if kt < 8 else KyT[t]
                        ks = (kt % 8) * 128
                        sc = mm_ps.tile([128, 1024], FP32, tag="mm", name="sc")
                        for hh in range(2):
                            nc.tensor.matmul(
                                sc[:, hh * 512:(hh + 1) * 512],
                                KT[hh * 64:(hh + 1) * 64, ks:ks + 128],
                                QT[t][hh * 64:(hh + 1) * 64, qt * 512:(qt + 1) * 512],
                                start=True,
                                stop=True,
                            )
                        p2 = ppool.tile([128, 1024], BF16, tag="p", name="p")
                        nc.scalar.activation(out=p2[:, :], in_=sc[:, :], func=EXP)
                        if (t, qt) in qk_fillers and kt in (4, 11):
                            fpi, fft = qk_fillers[(t, qt)]
                            emit_qk_half(fpi, fft, 0 if kt == 4 else 1)
                        # AV trails the exp stream by 2 groups so the PE
                        # never head-of-line blocks on an in-flight exp
                        if kt >= 2:
                            for hh in range(2):
                                nc.tensor.matmul(
                                    o_ps[hh][0:DH + 1, :],
                                    V[kt - 2][:, 2 * t + hh, :],
                                    hist[kt - 2][:, hh * 512:(hh + 1) * 512],
                                    start=(kt == 2),
                                    stop=False,
                                )
                        hist.append(p2)
                    for kt_tail in (14, 15):
                        for hh in range(2):
                            nc.tensor.matmul(
                                o_ps[hh][0:DH + 1, :],
                                V[kt_tail][:, 2 * t + hh, :],
                                hist[kt_tail][:, hh * 512:(hh + 1) * 512],
                                start=False,
                                stop=(kt_tail == 15),
                            )
                    if pending is not None:
                        emit_finalize(*pending)
                        pending = None
                    is_last = (t == 3 and qt == 1)
                    o_sb = []
                    rfs = []
                    recips = []
                    # den copies + reciprocals first so the (last-group) PE
                    # broadcast can start as early as possible
                    for i in range(2):
                        s2h = spool.tile([1, 512], FP32, tag="s2", name="s2h")
                        nc.vector.tensor_copy(out=s2h[:, :], in_=o_ps[i][DH:DH + 1, :])
                        rfh = spool.tile([1, 512], FP32, tag="recipf", name="rfh")
                        nc.vector.reciprocal_approx_fast(out=rfh[:, :], in_=s2h[:, :])
                        rfs.append(rfh)
                    for i in range(2):
                        ob = spool.tile([DH, 512], FP32, tag="osb", name="osb")
                        nc.vector.tensor_copy(out=ob[:, :], in_=o_ps[i][0:DH, :])
                        o_sb.append(ob)
                        if not is_last:
                            rd = dpool.tile([1, 512], FP32, name="rd")
                            nc.gpsimd.dma_start(out=rd[:, :], in_=rfs[i][0:1, :])
                            recips.append(rd)
                    if is_last:
                        last_fast = (t, qt, o_sb, rfs)
                    else:
                        pending = (t, qt, o_sb, recips)
            if pending is not None:
                emit_finalize(*pending)
            emit_finalize_fast(*last_fast)

            # ---- out-projection (transposed domain [m, s]) ----
            # 3-deep software pipeline over the 8 m-tiles; ft0-2 first, ft3
            # (gated on the last head-pair's finalize) as late as possible.
            def op_mms(ps, mt, fts):
                for ft in fts:
                    for half in range(2):
                        nc.tensor.matmul(
                            ps[:, half * 512:(half + 1) * 512],
                            wo_sb[ft][:, mt * 128:(mt + 1) * 128],
                            oT[ft][:, half * 512:(half + 1) * 512],
                            start=(ft == 0),
                            stop=(ft == 3),
                        )

            def op_finish(ps, mt):
                osb = opool.tile([128, 1024], FP16, tag="osb2", name="osb2")
                # alternate the PSUM->SBUF cast between DVE and ACT so the
                # tail isn't serialized on one engine; split the store over
                # two DMA rings so the last transfer clears quickly
                if mt % 2 == 0:
                    nc.vector.tensor_copy(out=osb[:, :], in_=ps[:, :])
                    eng = nc.sync
                else:
                    nc.scalar.copy(out=osb[:, :], in_=ps[:, :])
                    eng = nc.scalar
                eng.dma_start(out=outT[mt * 128:(mt + 1) * 128, 0:512],
                              in_=osb[:, 0:512])
                eng.dma_start(out=outT[mt * 128:(mt + 1) * 128, 512:1024],
                              in_=osb[:, 512:1024])

            op_tiles = {}
            for mt in range(3):
                op_tiles[mt] = mm_ps.tile([128, 1024], FP32, tag="mm", name=f"obs{mt}")
                op_mms(op_tiles[mt], mt, range(3))
            for mt in range(8):
                op_mms(op_tiles[mt], mt, [3])
                op_finish(op_tiles[mt], mt)
                nxt = mt + 3
                if nxt < 8:
                    op_tiles[nxt] = mm_ps.tile([128, 1024], FP32, tag="mm", name=f"obs{nxt}")
                    op_mms(op_tiles[nxt], nxt, range(3))

    # populate .instr bytes for extended-inst InstISA subclasses (the
    # custom-DVE reciprocal) — raw Bass skips this pass and the NEFF
    # compiler errors with "ISA wrong length" without it
    mybir.codegen_inst_isa_subclasses(nc)
    _spill_excess_waits(nc)
    return nc


_NC = None


def _get_program():
    global _NC
    if _NC is None:
        _NC = _build_program()
    return _NC


# ---------------------------------------------------------------------------
# host wrapper
# ---------------------------------------------------------------------------

def _prep_in_maps(x, y, W_Kx, b_Kx, W_Qx, b_Qx, W_Vx, b_Vx, W_Ky, b_Ky,
                  W_Vy, b_Vy, W_out, b_out):
    f32 = np.float32
    bf16 = ml_dtypes.bfloat16
    in_maps = []
    for c in range(NCORES):
        b = c // 2
        g = c % 2
        gs = slice(FS * g, FS * (g + 1))
        m = {
            "xT": np.ascontiguousarray(np.asarray(x[b], f32).T).astype(bf16),
            "yT": np.ascontiguousarray(np.asarray(y[b], f32).T).astype(bf16),
            "wq": np.ascontiguousarray((np.asarray(W_Qx, f32)[gs, :] / 8.0).T).astype(bf16),
            "wkx": np.ascontiguousarray(np.asarray(W_Kx, f32)[gs, :].T).astype(bf16),
            "wky": np.ascontiguousarray(np.asarray(W_Ky, f32)[gs, :].T).astype(bf16),
            "wvx": np.ascontiguousarray(np.asarray(W_Vx, f32)[gs, :].T).astype(bf16),
            "wvy": np.ascontiguousarray(np.asarray(W_Vy, f32)[gs, :].T).astype(bf16),
            "wo": np.ascontiguousarray(np.asarray(W_out, f32)[:, gs].T).astype(bf16),
            "bq": np.ascontiguousarray(
                (np.asarray(b_Qx, f32)[gs] / 8.0).reshape(4, 128).T),
            "bkx": np.ascontiguousarray(np.asarray(b_Kx, f32)[gs].reshape(4, 128).T),
            "bky": np.ascontiguousarray(np.asarray(b_Ky, f32)[gs].reshape(4, 128).T),
            "bvx_bc": np.ascontiguousarray(np.asarray(b_Vx, f32)[gs].reshape(1, FS)),
            "bvy_bc": np.ascontiguousarray(np.asarray(b_Vy, f32)[gs].reshape(1, FS)),
        }
        in_maps.append(m)
    return in_maps


def _assemble(results, b_out):
    B = 4
    out = np.empty((B, S, DIM), np.float32)
    bo = np.asarray(b_out, np.float32)
    for b in range(B):
        acc = (results[2 * b]["outT"].astype(np.float32)
               + results[2 * b + 1]["outT"].astype(np.float32))
        out[b] = acc.T + bo
    return out


def kernel(**inputs):
    nc = _get_program()
    in_maps = _prep_in_maps(**inputs)
    last_err = None
    for _attempt in range(3):
        try:
            res = run_bass_kernel_spmd(nc, in_maps, core_ids=list(range(NCORES)))
            return _assemble(res.results, inputs["b_out"])
        except Exception as e:  # transient NRT_EXEC_UNIT_UNRECOVERABLE after fresh compile
            last_err = e
            import time as _time
            _time.sleep(2.0)
    raise last_err


def kernel_traced(trace_cores=None, **inputs):
    """Same as kernel() but returns (out, BassKernelResults) with NTFF trace."""
    _register_ntff_hook()
    nc = _get_program()
    in_maps = _prep_in_maps(**inputs)
    res = run_bass_kernel_spmd(
        nc, in_maps, core_ids=list(range(NCORES)), trace=True,
        trace_cores=trace_cores or [0],
    )
    return _assemble(res.results, inputs["b_out"]), res


# revision 25
# speedup vs baseline: 1.1397x; 1.0062x over previous
"""MultiHeadCrossAttention kernel for 8 Trainium2 NeuronCores.

Problem (hardcoded): B=4, Sx=Sy=1024, DIM=1024, H=16, Dh=64, fp32.
  Q = x@W_Qx.T+b_Qx ; K = cat(x@W_Kx.T+b_Kx, y@W_Ky.T+b_Ky) per head
  V = cat(x@W_Vx.T+b_Vx, y@W_Vy.T+b_Vy) ; out = softmax(QK^T/8)V @ W_out.T + b_out

Sharding: core c -> (batch b = c//2, head-group g = c%2 of 8 heads).
Each core computes its batch's attention for its 8 heads plus the partial
out-projection over its 512 features; host sums the two partials per batch
and adds b_out (the "all-reduce after to_out", done in the gather).

Device layout choices (all matmuls natural, zero on-device transposes):
 - activations pre-transposed on host: xT/yT [dim, seq]
 - Q/K projections in transposed domain [feat, seq]  (bias = per-partition)
 - V in natural domain [seq, feat] with host-broadcast bias, plus a ones
   column per head -> AV matmul row 64 yields the softmax denominator
 - scoresT [k, q] via lhsT=KT (d=64 contraction; head pairs row-pack the PE)
 - exp on ACT only (no max subtraction: |scores| <~ 3), normalize via
   PE-broadcast reciprocal, out-projection in transposed domain [m, s]

v2 scheduling: DMA loads split into priority slices and issued across 4
engine queues (first V matmul ~4us instead of 13); reciprocal via the
fast approx custom-DVE op; out-projection fully 3-deep pipelined right
after the last AV matmul; fp16 partial outputs (halves the output DMA).
"""

import os
import sys

os.environ.setdefault("MYCRO_LOCAL_CACHE", "1")
if "/opt/trn_rl_repo" not in sys.path:
    sys.path.insert(0, "/opt/trn_rl_repo")

import ml_dtypes
import numpy as np

import concourse.bass as bass
import concourse.mybir as mybir
import concourse.tile as tile
from concourse import bass_utils
from concourse.bass_utils import run_bass_kernel_spmd

FP32 = mybir.dt.float32
FP16 = mybir.dt.float16
BF16 = mybir.dt.bfloat16

DIM = 1024
H = 16          # total heads
HG = 8          # heads per core (head-group)
DH = 64
S = 1024        # Sx = Sy
FS = 512        # feature slice per core (HG * DH)
NCORES = 8

# ---------------------------------------------------------------------------
# harness patches (this snapshot's Tile emits >1 wait per instruction in a
# few places; HW instructions hold one wait)
# ---------------------------------------------------------------------------

def _patched_drain_and_barrier(self, tick_clock, wait_clock):
    from bass_rust import ScopedClock

    nc = self.nc
    drain_inst = nc.sync.drain()
    wait_clock.add_sem_waits(
        drain_inst.ins, ScopedClock({None: tick_clock.global_clock})
    )
    si = drain_inst.ins.sync_info
    waits = list(si.on_wait)
    if len(waits) > 1:
        del si.on_wait[1:]
        for w in waits[1:]:
            nop = nc.sync.nop(nofuse=True, hint="drain_wait_spill")
            if nop.ins.sync_info is None:
                nop.ins.sync_info = mybir.SyncInfo(on_wait=[], on_update=[])
            nop.ins.sync_info.on_wait.append(w)

    nc.all_engine_barrier()
    assert self.sems is not None
    popped = nc._tile_sem_poison_stack.pop()
    assert popped is self._sem_poison
    nc.clear_and_free_semaphores(list(self.sems.allocated().values()))
    nc.all_engine_barrier()


def _spill_excess_waits(nc):
    n = 0
    for fn in nc.m.functions:
        for bb in fn.blocks:
            new_insts = []
            for inst in bb.instructions:
                si = getattr(inst, "sync_info", None)
                cap = 2 if isinstance(inst, mybir.InstEventSemaphore) else 1
                if si is not None and si.on_wait and len(si.on_wait) > cap:
                    extras = list(si.on_wait[cap:])
                    del si.on_wait[cap:]
                    for w in extras:
                        new_insts.append(
                            mybir.InstNoOp(
                                name=f"wspill-{nc.next_id()}",
                                engine=inst.engine,
                                ins=[],
                                outs=[],
                                sync_info=mybir.SyncInfo(on_wait=[w], on_update=[]),
                            )
                        )
                        n += 1
                new_insts.append(inst)
            bb.instructions[:] = new_insts
    return n


tile.TileContext._drain_and_barrier = _patched_drain_and_barrier

bass_utils.upload_artifacts = lambda tmpdir: tmpdir  # no S3 in container


def _register_ntff_hook():
    """Best-effort: enables trace=True runs (used by test harness only)."""
    try:
        try:
            from antenv.axon_hooks import set_axon_ntff_profile_hook
        except ImportError:
            # this container's antenv lacks axon_hooks — synthesize it
            import types

            import antenv

            mod = types.ModuleType("antenv.axon_hooks")
            _hook = [None]
            mod.set_axon_ntff_profile_hook = lambda h: _hook.__setitem__(0, h)
            mod.get_axon_ntff_profile_hook = lambda: _hook[0]
            sys.modules["antenv.axon_hooks"] = mod
            antenv.axon_hooks = mod
            set_axon_ntff_profile_hook = mod.set_axon_ntff_profile_hook
        sys.path.insert(0, "/root/.axon_site")
        from trn_agent_boot.trn_boot import _ntff_profile_via_ctypes

        set_axon_ntff_profile_hook(
            _ntff_profile_via_ctypes("/opt/axon/libaxon_pjrt.so")
        )
    except Exception:
        pass


# ---------------------------------------------------------------------------
# device program (identical on all 8 cores; per-core data differs)
# ---------------------------------------------------------------------------

def _build_program():
    nc = bass.Bass()

    xT = nc.declare_dram_parameter("xT", [DIM, S], BF16, isOutput=False)
    yT = nc.declare_dram_parameter("yT", [DIM, S], BF16, isOutput=False)
    wq = nc.declare_dram_parameter("wq", [DIM, FS], BF16, isOutput=False)
    wkx = nc.declare_dram_parameter("wkx", [DIM, FS], BF16, isOutput=False)
    wky = nc.declare_dram_parameter("wky", [DIM, FS], BF16, isOutput=False)
    wvx = nc.declare_dram_parameter("wvx", [DIM, FS], BF16, isOutput=False)
    wvy = nc.declare_dram_parameter("wvy", [DIM, FS], BF16, isOutput=False)
    wo = nc.declare_dram_parameter("wo", [FS, DIM], BF16, isOutput=False)
    bq = nc.declare_dram_parameter("bq", [128, 4], FP32, isOutput=False)
    bkx = nc.declare_dram_parameter("bkx", [128, 4], FP32, isOutput=False)
    bky = nc.declare_dram_parameter("bky", [128, 4], FP32, isOutput=False)
    bvx_bc = nc.declare_dram_parameter("bvx_bc", [1, FS], FP32, isOutput=False)
    bvy_bc = nc.declare_dram_parameter("bvy_bc", [1, FS], FP32, isOutput=False)
    outT = nc.declare_dram_parameter("outT", [DIM, S], FP16, isOutput=True)

    EXP = mybir.ActivationFunctionType.Exp

    with tile.TileContext(nc) as tc:
        import contextlib

        with contextlib.ExitStack() as ctx:
            big = ctx.enter_context(tc.tile_pool(name="big", bufs=24))
            wpool = ctx.enter_context(tc.tile_pool(name="wpool", bufs=40))
            qkv = ctx.enter_context(tc.tile_pool(name="qkv", bufs=12))
            vpool = ctx.enter_context(tc.tile_pool(name="vpool", bufs=16))
            ppool = ctx.enter_context(tc.tile_pool(name="ppool", bufs=6))
            opool = ctx.enter_context(tc.tile_pool(name="opool", bufs=4))
            spool = ctx.enter_context(tc.tile_pool(name="spool", bufs=4))
            cpool = ctx.enter_context(tc.tile_pool(name="cpool", bufs=1))
            dpool = ctx.enter_context(tc.tile_pool(name="dpool", bufs=8, space="DRAM"))
            mm_ps = ctx.enter_context(tc.tile_pool(name="mm_ps", bufs=3, space="PSUM"))
            ot_ps = ctx.enter_context(tc.tile_pool(name="ot_ps", bufs=2, space="PSUM"))

            # ---- constants ----
            ones_f32 = cpool.tile([128, 64], FP32, tag="ones_f32")
            nc.vector.memset(ones_f32[:, :], 1.0)
            bq_sb = cpool.tile([128, 4], FP32, tag="bq")
            bkx_sb = cpool.tile([128, 4], FP32, tag="bkx")
            bky_sb = cpool.tile([128, 4], FP32, tag="bky")
            bvx_sb = cpool.tile([128, FS], FP32, tag="bvx")
            bvy_sb = cpool.tile([128, FS], FP32, tag="bvy")

            def _bcast_ap(h):
                return bass.AP(
                    tensor=h[:, :].tensor, offset=h[:, :].offset,
                    ap=[[0, 128]] + [list(a) for a in h[:, :].ap[1:]],
                )

            # ---- tile allocations for activations + weights ----
            xt = [big.tile([128, S], BF16, tag="big", name=f"xt{i}") for i in range(8)]
            yt = [big.tile([128, S], BF16, tag="big", name=f"yt{i}") for i in range(8)]
            wo_sb = [big.tile([128, S], BF16, tag="big", name=f"wo{i}") for i in range(4)]
            wvx_sb = [wpool.tile([128, FS], BF16, tag="w", name=f"wvx{i}") for i in range(8)]
            wvy_sb = [wpool.tile([128, FS], BF16, tag="w", name=f"wvy{i}") for i in range(8)]
            qk_w = [[wpool.tile([128, FS], BF16, tag="w", name=f"wp{pi}_{ct}")
                     for ct in range(8)] for pi in range(3)]  # Q, Kx, Ky

            # ---- DMA issues: single-queue priority order (HBM bandwidth is
            # shared, so the critical V-x path must not compete with later
            # loads — parallel queues made it WORSE). Baseline order, with
            # the first xt chunk split across two rings for an earlier
            # first matmul.
            for ct in range(8):
                nc.sync.dma_start(out=wvx_sb[ct], in_=wvx[ct * 128:(ct + 1) * 128, :])
                nc.sync.dma_start(out=xt[ct], in_=xT[ct * 128:(ct + 1) * 128, :])
            for ct in range(8):
                nc.sync.dma_start(out=wvy_sb[ct], in_=wvy[ct * 128:(ct + 1) * 128, :])
                nc.sync.dma_start(out=yt[ct], in_=yT[ct * 128:(ct + 1) * 128, :])
            for ct in range(8):
                nc.sync.dma_start(out=qk_w[0][ct], in_=wq[ct * 128:(ct + 1) * 128, :])
            for ct in range(8):
                nc.sync.dma_start(out=qk_w[1][ct], in_=wkx[ct * 128:(ct + 1) * 128, :])
            for ct in range(8):
                nc.sync.dma_start(out=qk_w[2][ct], in_=wky[ct * 128:(ct + 1) * 128, :])
            for ft in range(4):
                nc.sync.dma_start(out=wo_sb[ft], in_=wo[ft * 128:(ft + 1) * 128, :])

            nc.gpsimd.dma_start(out=bq_sb, in_=bq[:, :])
            nc.gpsimd.dma_start(out=bkx_sb, in_=bkx[:, :])
            nc.gpsimd.dma_start(out=bky_sb, in_=bky[:, :])
            nc.gpsimd.dma_start(out=bvx_sb, in_=_bcast_ap(bvx_bc))
            nc.gpsimd.dma_start(out=bvy_sb, in_=_bcast_ap(bvy_bc))

            # ---- V projections (natural domain, bias + ones column) ----
            V = [vpool.tile([128, HG, DH + 1], BF16, tag="v", name=f"V{i}") for i in range(16)]
            for src_is_y in (False, True):
                bias_sb = bvy_sb if src_is_y else bvx_sb
                base = 8 if src_is_y else 0
                w_sb = wvy_sb if src_is_y else wvx_sb
                act = yt if src_is_y else xt
                for sg in range(4):  # two s-tiles per psum group
                    ps = mm_ps.tile([128, 1024], FP32, tag="mm", name="vps")
                    for ct in range(8):
                        for half in range(2):
                            st = 2 * sg + half
                            nc.tensor.matmul(
                                ps[:, half * 512:(half + 1) * 512],
                                act[ct][:, st * 128:(st + 1) * 128],
                                w_sb[ct][:, :],
                                start=(ct == 0),
                                stop=(ct == 7),
                            )
                    for half in range(2):
                        st = 2 * sg + half
                        vt = V[base + st]
                        nc.vector.tensor_add(
                            out=vt[:, :, 0:DH],
                            in0=ps[:, half * 512:(half + 1) * 512].rearrange(
                                "p (h d) -> p h d", h=HG),
                            in1=bias_sb[:, :].rearrange("p (h d) -> p h d", h=HG),
                        )
                        nc.vector.tensor_copy(
                            out=vt[:, :, DH:DH + 1],
                            in_=ones_f32[:, 0:HG].rearrange("p (h o) -> p h o", o=1),
                        )

            # ---- Q/K projections (transposed domain [feat, seq]) ----
            QT = [qkv.tile([128, S], BF16, tag="qkv", name=f"QT{i}") for i in range(4)]
            KxT = [qkv.tile([128, S], BF16, tag="qkv", name=f"KxT{i}") for i in range(4)]
            KyT = [qkv.tile([128, S], BF16, tag="qkv", name=f"KyT{i}") for i in range(4)]

            qk_act = [xt, xt, yt]
            qk_bias = [bq_sb, bkx_sb, bky_sb]
            qk_dst = [QT, KxT, KyT]
            qk_ps = {}

            def emit_qk_half(pi, ft, half):
                key = (pi, ft)
                if key not in qk_ps:
                    qk_ps[key] = mm_ps.tile(
                        [128, 1024], FP32, tag="mm", name=f"qkps{pi}_{ft}"
                    )
                ps = qk_ps[key]
                w_sb = qk_w[pi]
                act = qk_act[pi]
                for ct in (range(0, 4) if half == 0 else range(4, 8)):
                    for h2 in range(2):
                        nc.tensor.matmul(
                            ps[:, h2 * 512:(h2 + 1) * 512],
                            w_sb[ct][:, ft * 128:(ft + 1) * 128],
                            act[ct][:, h2 * 512:(h2 + 1) * 512],
                            start=(ct == 0),
                            stop=(ct == 7),
                        )
                if half == 1:
                    nc.vector.tensor_scalar_add(
                        out=qk_dst[pi][ft][:, :],
                        in0=ps[:, :],
                        scalar1=qk_bias[pi][:, ft:ft + 1],
                    )
                    del qk_ps[key]

            def emit_qk_single(pi, ft, j):
                """One filler matmul (ct=j//2, h2=j%2) — spread so each
                attention group carries ~216ns of projection work instead
                of 1.7us bursts that stall the exp stream."""
                key = (pi, ft)
                if key not in qk_ps:
                    qk_ps[key] = mm_ps.tile(
                        [128, 1024], FP32, tag="mm", name=f"qkps{pi}_{ft}"
                    )
                ps = qk_ps[key]
                ct, h2 = j // 2, j % 2
                nc.tensor.matmul(
                    ps[:, h2 * 512:(h2 + 1) * 512],
                    qk_w[pi][ct][:, ft * 128:(ft + 1) * 128],
                    qk_act[pi][ct][:, h2 * 512:(h2 + 1) * 512],
                    start=(ct == 0),
                    stop=(ct == 7),
                )
                if j == 15:
                    nc.vector.tensor_scalar_add(
                        out=qk_dst[pi][ft][:, :],
                        in0=ps[:, :],
                        scalar1=qk_bias[pi][:, ft:ft + 1],
                    )
                    del qk_ps[key]

            # upfront: ft=0 for all projections, plus all fts of proj 2 (wky)
            for pi in range(3):
                emit_qk_half(pi, 0, 0)
                emit_qk_half(pi, 0, 1)
            for ft in range(1, 4):
                emit_qk_half(2, ft, 0)
                emit_qk_half(2, ft, 1)

            # remaining 6 ft-groups of Q/Kx ride the attention phase's spare
            # PE cycles, one matmul per (kt) group
            qk_fillers = {(t, qt): (qt, t + 1) for t in range(3) for qt in range(2)}

            # ---- attention (head pairs row-pack; both q-tiles share one
            #      psum tile so exp runs once per (kt, head)) ----
            oT = [big.tile([128, S], BF16, tag="big", name=f"oT{i}") for i in range(4)]

            def emit_finalize(t, qt, o_sb, recips):
                for hh in range(2):
                        i = hh
                        rd = recips[i]
                        bc_sb = spool.tile([DH, 512], FP32, tag="bc", name="bc_sb")
                        rd_bcast = bass.AP(
                            tensor=rd.tensor, offset=rd.offset,
                            ap=[[0, DH]] + [list(a) for a in rd.ap[1:]],
                        )
                        nc.gpsimd.dma_start(out=bc_sb[:, :], in_=rd_bcast)
                        nc.vector.tensor_mul(
                            out=oT[t][hh * 64:hh * 64 + DH, qt * 512:(qt + 1) * 512],
                            in0=o_sb[i][:, :],
                            in1=bc_sb[:, :],
                        )

            def emit_finalize_fast(t, qt, o_sb, rfs):
                # last-group variant: broadcast the reciprocal via a K=1
                # PE matmul into a just-freed PSUM bank instead of the DRAM
                # bounce — ~2us chain instead of ~7us, and the PE never
                # idles long enough for HAM to re-throttle.
                for hh in range(2):
                    bc_ps = ot_ps.tile([128, 512], FP32, tag="ot", name="bc_ps")
                    nc.tensor.matmul(
                        bc_ps[0:DH, :],
                        ones_f32[0:1, 0:DH],
                        rfs[hh][0:1, :],
                        start=True,
                        stop=True,
                    )
                    nc.vector.tensor_mul(
                        out=oT[t][hh * 64:hh * 64 + DH, qt * 512:(qt + 1) * 512],
                        in0=o_sb[hh][:, :],
                        in1=bc_ps[0:DH, :],
                    )

            pending = None
            for t in range(4):  # heads 2t, 2t+1
                for qt in range(2):
                    o_ps = [ot_ps.tile([128, 512], FP32, tag="ot", name=f"ops{i}")
                            for i in range(2)]  # per head of the pair
                    hist = []
                    for kt in range(16):
                        KT = KxT[t] if kt < 8 else KyT[t]
                        ks = (kt % 8) * 128
                        sc = mm_ps.tile([128, 1024], FP32, tag="mm", name="sc")
                        for hh in range(2):
                            nc.tensor.matmul(
                                sc[:, hh * 512:(hh + 1) * 512],
                                KT[hh * 64:(hh + 1) * 64, ks:ks + 128],
                                QT[t][hh * 64:(hh + 1) * 64, qt * 512:(qt + 1) * 512],
                                start=True,
                                stop=True,
                            )
                        p2 = ppool.tile([128, 1024], BF16, tag="p", name="p")
                        nc.scalar.activation(out=p2[:, :], in_=sc[:, :], func=EXP)
                        # AV trails the exp stream by 2 groups so the PE
                        # never head-of-line blocks on an in-flight exp
                        if kt >= 2:
                            for hh in range(2):
                                nc.tensor.matmul(
                                    o_ps[hh][0:DH + 1, :],
                                    V[kt - 2][:, 2 * t + hh, :],
                                    hist[kt - 2][:, hh * 512:(hh + 1) * 512],
                                    start=(kt == 2),
                                    stop=False,
                                )
                        # filler after the AVs so its LDWEIGHTS hides behind
                        # the full-array AV matmuls
                        if (t, qt) in qk_fillers:
                            fpi, fft = qk_fillers[(t, qt)]
                            emit_qk_single(fpi, fft, kt)
                        hist.append(p2)
                    for kt_tail in (14, 15):
                        for hh in range(2):
                            nc.tensor.matmul(
                                o_ps[hh][0:DH + 1, :],
                                V[kt_tail][:, 2 * t + hh, :],
                                hist[kt_tail][:, hh * 512:(hh + 1) * 512],
                                start=False,
                                stop=(kt_tail == 15),
                            )
                    is_last = (t == 3 and qt == 1)
                    o_sb = []
                    rfs = []
                    recips = []
                    # den copies + reciprocals first so the (last-group) PE
                    # broadcast can start as early as possible; the previous
                    # group's finalize muls are emitted only after these
                    # copies so the o_ps banks free up without DVE
                    # head-of-line blocking
                    for i in range(2):
                        s2h = spool.tile([1, 512], FP32, tag="s2", name="s2h")
                        nc.vector.tensor_copy(out=s2h[:, :], in_=o_ps[i][DH:DH + 1, :])
                        rfh = spool.tile([1, 512], FP32, tag="recipf", name="rfh")
                        nc.vector.reciprocal_approx_fast(out=rfh[:, :], in_=s2h[:, :])
                        rfs.append(rfh)
                    for i in range(2):
                        ob = spool.tile([DH, 512], FP32, tag="osb", name="osb")
                        nc.vector.tensor_copy(out=ob[:, :], in_=o_ps[i][0:DH, :])
                        o_sb.append(ob)
                        if not is_last:
                            rd = dpool.tile([1, 512], FP32, name="rd")
                            nc.gpsimd.dma_start(out=rd[:, :], in_=rfs[i][0:1, :])
                            recips.append(rd)
                    if pending is not None:
                        emit_finalize(*pending)
                        pending = None
                    if is_last:
                        last_fast = (t, qt, o_sb, rfs)
                    else:
                        pending = (t, qt, o_sb, recips)
            if pending is not None:
                emit_finalize(*pending)
            emit_finalize_fast(*last_fast)

            # ---- out-projection (transposed domain [m, s]) ----
            # 3-deep software pipeline over the 8 m-tiles; ft0-2 first, ft3
            # (gated on the last head-pair's finalize) as late as possible.
            def op_mms(ps, mt, fts):
                for ft in fts:
                    for half in range(2):
                        nc.tensor.matmul(
                            ps[:, half * 512:(half + 1) * 512],
                            wo_sb[ft][:, mt * 128:(mt + 1) * 128],
                            oT[ft][:, half * 512:(half + 1) * 512],
                            start=(ft == 0),
                            stop=(ft == 3),
                        )

            def op_finish(ps, mt):
                osb = opool.tile([128, 1024], FP16, tag="osb2", name="osb2")
                # alternate the PSUM->SBUF cast between DVE and ACT so the
                # tail isn't serialized on one engine; split the store over
                # two DMA rings so the last transfer clears quickly
                if mt % 2 == 0:
                    nc.vector.tensor_copy(out=osb[:, :], in_=ps[:, :])
                    eng = nc.sync
                else:
                    nc.scalar.copy(out=osb[:, :], in_=ps[:, :])
                    eng = nc.scalar
                eng.dma_start(out=outT[mt * 128:(mt + 1) * 128, 0:512],
                              in_=osb[:, 0:512])
                eng.dma_start(out=outT[mt * 128:(mt + 1) * 128, 512:1024],
                              in_=osb[:, 512:1024])

            op_tiles = {}
            for mt in range(3):
                op_tiles[mt] = mm_ps.tile([128, 1024], FP32, tag="mm", name=f"obs{mt}")
                op_mms(op_tiles[mt], mt, range(3))
            for mt in range(8):
                op_mms(op_tiles[mt], mt, [3])
                op_finish(op_tiles[mt], mt)
                nxt = mt + 3
                if nxt < 8:
                    op_tiles[nxt] = mm_ps.tile([128, 1024], FP32, tag="mm", name=f"obs{nxt}")
                    op_mms(op_tiles[nxt], nxt, range(3))

    # populate .instr bytes for extended-inst InstISA subclasses (the
    # custom-DVE reciprocal) — raw Bass skips this pass and the NEFF
    # compiler errors with "ISA wrong length" without it
    mybir.codegen_inst_isa_subclasses(nc)
    _spill_excess_waits(nc)
    return nc


_NC = None


def _get_program():
    global _NC
    if _NC is None:
        _NC = _build_program()
    return _NC


# ---------------------------------------------------------------------------
# host wrapper
# ---------------------------------------------------------------------------

def _prep_in_maps(x, y, W_Kx, b_Kx, W_Qx, b_Qx, W_Vx, b_Vx, W_Ky, b_Ky,
                  W_Vy, b_Vy, W_out, b_out):
    f32 = np.float32
    bf16 = ml_dtypes.bfloat16
    in_maps = []
    for c in range(NCORES):
        b = c // 2
        g = c % 2
        gs = slice(FS * g, FS * (g + 1))
        m = {
            "xT": np.ascontiguousarray(np.asarray(x[b], f32).T).astype(bf16),
            "yT": np.ascontiguousarray(np.asarray(y[b], f32).T).astype(bf16),
            "wq": np.ascontiguousarray((np.asarray(W_Qx, f32)[gs, :] / 8.0).T).astype(bf16),
            "wkx": np.ascontiguousarray(np.asarray(W_Kx, f32)[gs, :].T).astype(bf16),
            "wky": np.ascontiguousarray(np.asarray(W_Ky, f32)[gs, :].T).astype(bf16),
            "wvx": np.ascontiguousarray(np.asarray(W_Vx, f32)[gs, :].T).astype(bf16),
            "wvy": np.ascontiguousarray(np.asarray(W_Vy, f32)[gs, :].T).astype(bf16),
            "wo": np.ascontiguousarray(np.asarray(W_out, f32)[:, gs].T).astype(bf16),
            "bq": np.ascontiguousarray(
                (np.asarray(b_Qx, f32)[gs] / 8.0).reshape(4, 128).T),
            "bkx": np.ascontiguousarray(np.asarray(b_Kx, f32)[gs].reshape(4, 128).T),
            "bky": np.ascontiguousarray(np.asarray(b_Ky, f32)[gs].reshape(4, 128).T),
            "bvx_bc": np.ascontiguousarray(np.asarray(b_Vx, f32)[gs].reshape(1, FS)),
            "bvy_bc": np.ascontiguousarray(np.asarray(b_Vy, f32)[gs].reshape(1, FS)),
        }
        in_maps.append(m)
    return in_maps


def _assemble(results, b_out):
    B = 4
    out = np.empty((B, S, DIM), np.float32)
    bo = np.asarray(b_out, np.float32)
    for b in range(B):
        acc = (results[2 * b]["outT"].astype(np.float32)
               + results[2 * b + 1]["outT"].astype(np.float32))
        out[b] = acc.T + bo
    return out


def kernel(**inputs):
    nc = _get_program()
    in_maps = _prep_in_maps(**inputs)
    last_err = None
    for _attempt in range(3):
        try:
            res = run_bass_kernel_spmd(nc, in_maps, core_ids=list(range(NCORES)))
            return _assemble(res.results, inputs["b_out"])
        except Exception as e:  # transient NRT_EXEC_UNIT_UNRECOVERABLE after fresh compile
            last_err = e
            import time as _time
            _time.sleep(2.0)
    raise last_err


def kernel_traced(trace_cores=None, **inputs):
    """Same as kernel() but returns (out, BassKernelResults) with NTFF trace."""
    _register_ntff_hook()
    nc = _get_program()
    in_maps = _prep_in_maps(**inputs)
    res = run_bass_kernel_spmd(
        nc, in_maps, core_ids=list(range(NCORES)), trace=True,
        trace_cores=trace_cores or [0],
    )
    return _assemble(res.results, inputs["b_out"]), res


# revision 27
# speedup vs baseline: 1.1794x; 1.0349x over previous
"""MultiHeadCrossAttention kernel for 8 Trainium2 NeuronCores.

Problem (hardcoded): B=4, Sx=Sy=1024, DIM=1024, H=16, Dh=64, fp32.
  Q = x@W_Qx.T+b_Qx ; K = cat(x@W_Kx.T+b_Kx, y@W_Ky.T+b_Ky) per head
  V = cat(x@W_Vx.T+b_Vx, y@W_Vy.T+b_Vy) ; out = softmax(QK^T/8)V @ W_out.T + b_out

Sharding: core c -> (batch b = c//2, head-group g = c%2 of 8 heads).
Each core computes its batch's attention for its 8 heads plus the partial
out-projection over its 512 features; host sums the two partials per batch
and adds b_out (the "all-reduce after to_out", done in the gather).

Device layout choices (all matmuls natural, zero on-device transposes):
 - activations pre-transposed on host: xT/yT [dim, seq]
 - Q/K projections in transposed domain [feat, seq]  (bias = per-partition)
 - V in natural domain [seq, feat] with host-broadcast bias, plus a ones
   column per head -> AV matmul row 64 yields the softmax denominator
 - scoresT [k, q] via lhsT=KT (d=64 contraction; head pairs row-pack the PE)
 - exp on ACT only (no max subtraction: |scores| <~ 3), normalize via
   PE-broadcast reciprocal, out-projection in transposed domain [m, s]

v2 scheduling: DMA loads split into priority slices and issued across 4
engine queues (first V matmul ~4us instead of 13); reciprocal via the
fast approx custom-DVE op; out-projection fully 3-deep pipelined right
after the last AV matmul; fp16 partial outputs (halves the output DMA).
"""

import os
import sys

os.environ.setdefault("MYCRO_LOCAL_CACHE", "1")
if "/opt/trn_rl_repo" not in sys.path:
    sys.path.insert(0, "/opt/trn_rl_repo")

import ml_dtypes
import numpy as np

import concourse.bass as bass
import concourse.mybir as mybir
import concourse.tile as tile
from concourse import bass_utils
from concourse.bass_utils import run_bass_kernel_spmd

FP32 = mybir.dt.float32
FP16 = mybir.dt.float16
BF16 = mybir.dt.bfloat16

DIM = 1024
H = 16          # total heads
HG = 8          # heads per core (head-group)
DH = 64
S = 1024        # Sx = Sy
FS = 512        # feature slice per core (HG * DH)
NCORES = 8

# ---------------------------------------------------------------------------
# harness patches (this snapshot's Tile emits >1 wait per instruction in a
# few places; HW instructions hold one wait)
# ---------------------------------------------------------------------------

def _patched_drain_and_barrier(self, tick_clock, wait_clock):
    from bass_rust import ScopedClock

    nc = self.nc
    drain_inst = nc.sync.drain()
    wait_clock.add_sem_waits(
        drain_inst.ins, ScopedClock({None: tick_clock.global_clock})
    )
    si = drain_inst.ins.sync_info
    waits = list(si.on_wait)
    if len(waits) > 1:
        del si.on_wait[1:]
        for w in waits[1:]:
            nop = nc.sync.nop(nofuse=True, hint="drain_wait_spill")
            if nop.ins.sync_info is None:
                nop.ins.sync_info = mybir.SyncInfo(on_wait=[], on_update=[])
            nop.ins.sync_info.on_wait.append(w)

    nc.all_engine_barrier()
    assert self.sems is not None
    popped = nc._tile_sem_poison_stack.pop()
    assert popped is self._sem_poison
    nc.clear_and_free_semaphores(list(self.sems.allocated().values()))
    nc.all_engine_barrier()


def _spill_excess_waits(nc):
    n = 0
    for fn in nc.m.functions:
        for bb in fn.blocks:
            new_insts = []
            for inst in bb.instructions:
                si = getattr(inst, "sync_info", None)
                cap = 2 if isinstance(inst, mybir.InstEventSemaphore) else 1
                if si is not None and si.on_wait and len(si.on_wait) > cap:
                    extras = list(si.on_wait[cap:])
                    del si.on_wait[cap:]
                    for w in extras:
                        new_insts.append(
                            mybir.InstNoOp(
                                name=f"wspill-{nc.next_id()}",
                                engine=inst.engine,
                                ins=[],
                                outs=[],
                                sync_info=mybir.SyncInfo(on_wait=[w], on_update=[]),
                            )
                        )
                        n += 1
                new_insts.append(inst)
            bb.instructions[:] = new_insts
    return n


tile.TileContext._drain_and_barrier = _patched_drain_and_barrier

bass_utils.upload_artifacts = lambda tmpdir: tmpdir  # no S3 in container


def _register_ntff_hook():
    """Best-effort: enables trace=True runs (used by test harness only)."""
    try:
        try:
            from antenv.axon_hooks import set_axon_ntff_profile_hook
        except ImportError:
            # this container's antenv lacks axon_hooks — synthesize it
            import types

            import antenv

            mod = types.ModuleType("antenv.axon_hooks")
            _hook = [None]
            mod.set_axon_ntff_profile_hook = lambda h: _hook.__setitem__(0, h)
            mod.get_axon_ntff_profile_hook = lambda: _hook[0]
            sys.modules["antenv.axon_hooks"] = mod
            antenv.axon_hooks = mod
            set_axon_ntff_profile_hook = mod.set_axon_ntff_profile_hook
        sys.path.insert(0, "/root/.axon_site")
        from trn_agent_boot.trn_boot import _ntff_profile_via_ctypes

        set_axon_ntff_profile_hook(
            _ntff_profile_via_ctypes("/opt/axon/libaxon_pjrt.so")
        )
    except Exception:
        pass


# ---------------------------------------------------------------------------
# device program (identical on all 8 cores; per-core data differs)
# ---------------------------------------------------------------------------

def _build_program():
    nc = bass.Bass()

    xT = nc.declare_dram_parameter("xT", [DIM, S], BF16, isOutput=False)
    yT = nc.declare_dram_parameter("yT", [DIM, S], BF16, isOutput=False)
    wq = nc.declare_dram_parameter("wq", [DIM, FS], BF16, isOutput=False)
    wkx = nc.declare_dram_parameter("wkx", [DIM, FS], BF16, isOutput=False)
    wky = nc.declare_dram_parameter("wky", [DIM, FS], BF16, isOutput=False)
    wvx = nc.declare_dram_parameter("wvx", [DIM, FS], BF16, isOutput=False)
    wvy = nc.declare_dram_parameter("wvy", [DIM, FS], BF16, isOutput=False)
    wo = nc.declare_dram_parameter("wo", [FS, DIM], BF16, isOutput=False)
    bq = nc.declare_dram_parameter("bq", [128, 4], FP32, isOutput=False)
    bkx = nc.declare_dram_parameter("bkx", [128, 4], FP32, isOutput=False)
    bky = nc.declare_dram_parameter("bky", [128, 4], FP32, isOutput=False)
    bvx_bc = nc.declare_dram_parameter("bvx_bc", [1, FS], FP32, isOutput=False)
    bvy_bc = nc.declare_dram_parameter("bvy_bc", [1, FS], FP32, isOutput=False)
    outT = nc.declare_dram_parameter("outT", [DIM, S], FP16, isOutput=True)

    EXP = mybir.ActivationFunctionType.Exp

    with tile.TileContext(nc) as tc:
        import contextlib

        with contextlib.ExitStack() as ctx:
            big = ctx.enter_context(tc.tile_pool(name="big", bufs=24))
            wpool = ctx.enter_context(tc.tile_pool(name="wpool", bufs=40))
            qkv = ctx.enter_context(tc.tile_pool(name="qkv", bufs=12))
            vpool = ctx.enter_context(tc.tile_pool(name="vpool", bufs=16))
            ppool = ctx.enter_context(tc.tile_pool(name="ppool", bufs=6))
            opool = ctx.enter_context(tc.tile_pool(name="opool", bufs=4))
            spool = ctx.enter_context(tc.tile_pool(name="spool", bufs=4))
            cpool = ctx.enter_context(tc.tile_pool(name="cpool", bufs=1))
            dpool = ctx.enter_context(tc.tile_pool(name="dpool", bufs=8, space="DRAM"))
            mm_ps = ctx.enter_context(tc.tile_pool(name="mm_ps", bufs=3, space="PSUM"))
            ot_ps = ctx.enter_context(tc.tile_pool(name="ot_ps", bufs=2, space="PSUM"))

            # ---- constants ----
            ones_f32 = cpool.tile([128, 64], FP32, tag="ones_f32")
            nc.vector.memset(ones_f32[:, :], 1.0)
            bq_sb = cpool.tile([128, 4], FP32, tag="bq")
            bkx_sb = cpool.tile([128, 4], FP32, tag="bkx")
            bky_sb = cpool.tile([128, 4], FP32, tag="bky")
            bvx_sb = cpool.tile([128, FS], FP32, tag="bvx")
            bvy_sb = cpool.tile([128, FS], FP32, tag="bvy")

            def _bcast_ap(h):
                return bass.AP(
                    tensor=h[:, :].tensor, offset=h[:, :].offset,
                    ap=[[0, 128]] + [list(a) for a in h[:, :].ap[1:]],
                )

            # ---- tile allocations for activations + weights ----
            xt = [big.tile([128, S], BF16, tag="big", name=f"xt{i}") for i in range(8)]
            yt = [big.tile([128, S], BF16, tag="big", name=f"yt{i}") for i in range(8)]
            wo_sb = [big.tile([128, S], BF16, tag="big", name=f"wo{i}") for i in range(4)]
            wvx_sb = [wpool.tile([128, FS], BF16, tag="w", name=f"wvx{i}") for i in range(8)]
            wvy_sb = [wpool.tile([128, FS], BF16, tag="w", name=f"wvy{i}") for i in range(8)]
            qk_w = [[wpool.tile([128, FS], BF16, tag="w", name=f"wp{pi}_{ct}")
                     for ct in range(8)] for pi in range(3)]  # Q, Kx, Ky

            # ---- DMA issues: single-queue priority order (HBM bandwidth is
            # shared, so the critical V-x path must not compete with later
            # loads — parallel queues made it WORSE). Baseline order, with
            # the first xt chunk split across two rings for an earlier
            # first matmul.
            for ct in range(8):
                nc.sync.dma_start(out=wvx_sb[ct], in_=wvx[ct * 128:(ct + 1) * 128, :])
                nc.sync.dma_start(out=xt[ct], in_=xT[ct * 128:(ct + 1) * 128, :])
            for ct in range(8):
                nc.sync.dma_start(out=wvy_sb[ct], in_=wvy[ct * 128:(ct + 1) * 128, :])
                nc.sync.dma_start(out=yt[ct], in_=yT[ct * 128:(ct + 1) * 128, :])
            for ct in range(8):
                nc.sync.dma_start(out=qk_w[0][ct], in_=wq[ct * 128:(ct + 1) * 128, :])
            for ct in range(8):
                nc.sync.dma_start(out=qk_w[1][ct], in_=wkx[ct * 128:(ct + 1) * 128, :])
            for ct in range(8):
                nc.sync.dma_start(out=qk_w[2][ct], in_=wky[ct * 128:(ct + 1) * 128, :])
            for ft in range(4):
                nc.sync.dma_start(out=wo_sb[ft], in_=wo[ft * 128:(ft + 1) * 128, :])

            nc.gpsimd.dma_start(out=bq_sb, in_=bq[:, :])
            nc.gpsimd.dma_start(out=bkx_sb, in_=bkx[:, :])
            nc.gpsimd.dma_start(out=bky_sb, in_=bky[:, :])
            nc.gpsimd.dma_start(out=bvx_sb, in_=_bcast_ap(bvx_bc))
            nc.gpsimd.dma_start(out=bvy_sb, in_=_bcast_ap(bvy_bc))

            # ---- V projections (natural domain, bias + ones column) ----
            V = [vpool.tile([128, HG, DH + 1], BF16, tag="v", name=f"V{i}") for i in range(16)]
            for src_is_y in (False, True):
                bias_sb = bvy_sb if src_is_y else bvx_sb
                base = 8 if src_is_y else 0
                w_sb = wvy_sb if src_is_y else wvx_sb
                act = yt if src_is_y else xt
                for sg in range(4):  # two s-tiles per psum group
                    ps = mm_ps.tile([128, 1024], FP32, tag="mm", name="vps")
                    for ct in range(8):
                        for half in range(2):
                            st = 2 * sg + half
                            nc.tensor.matmul(
                                ps[:, half * 512:(half + 1) * 512],
                                act[ct][:, st * 128:(st + 1) * 128],
                                w_sb[ct][:, :],
                                start=(ct == 0),
                                stop=(ct == 7),
                            )
                    for half in range(2):
                        st = 2 * sg + half
                        vt = V[base + st]
                        nc.vector.tensor_add(
                            out=vt[:, :, 0:DH],
                            in0=ps[:, half * 512:(half + 1) * 512].rearrange(
                                "p (h d) -> p h d", h=HG),
                            in1=bias_sb[:, :].rearrange("p (h d) -> p h d", h=HG),
                        )
                        nc.vector.tensor_copy(
                            out=vt[:, :, DH:DH + 1],
                            in_=ones_f32[:, 0:HG].rearrange("p (h o) -> p h o", o=1),
                        )

            # ---- Q/K projections (transposed domain [feat, seq]) ----
            QT = [qkv.tile([128, S], BF16, tag="qkv", name=f"QT{i}") for i in range(4)]
            KxT = [qkv.tile([128, S], BF16, tag="qkv", name=f"KxT{i}") for i in range(4)]
            KyT = [qkv.tile([128, S], BF16, tag="qkv", name=f"KyT{i}") for i in range(4)]

            qk_act = [xt, xt, yt]
            qk_bias = [bq_sb, bkx_sb, bky_sb]
            qk_dst = [QT, KxT, KyT]
            qk_ps = {}

            def emit_qk_half(pi, ft, half):
                key = (pi, ft)
                if key not in qk_ps:
                    qk_ps[key] = mm_ps.tile(
                        [128, 1024], FP32, tag="mm", name=f"qkps{pi}_{ft}"
                    )
                ps = qk_ps[key]
                w_sb = qk_w[pi]
                act = qk_act[pi]
                for ct in (range(0, 4) if half == 0 else range(4, 8)):
                    for h2 in range(2):
                        nc.tensor.matmul(
                            ps[:, h2 * 512:(h2 + 1) * 512],
                            w_sb[ct][:, ft * 128:(ft + 1) * 128],
                            act[ct][:, h2 * 512:(h2 + 1) * 512],
                            start=(ct == 0),
                            stop=(ct == 7),
                        )
                if half == 1:
                    nc.vector.tensor_scalar_add(
                        out=qk_dst[pi][ft][:, :],
                        in0=ps[:, :],
                        scalar1=qk_bias[pi][:, ft:ft + 1],
                    )
                    del qk_ps[key]

            def emit_qk_single(pi, ft, j):
                """One filler matmul (ct=j//2, h2=j%2) — spread so each
                attention group carries ~216ns of projection work instead
                of 1.7us bursts that stall the exp stream."""
                key = (pi, ft)
                if key not in qk_ps:
                    qk_ps[key] = mm_ps.tile(
                        [128, 1024], FP32, tag="mm", name=f"qkps{pi}_{ft}"
                    )
                ps = qk_ps[key]
                ct, h2 = j // 2, j % 2
                nc.tensor.matmul(
                    ps[:, h2 * 512:(h2 + 1) * 512],
                    qk_w[pi][ct][:, ft * 128:(ft + 1) * 128],
                    qk_act[pi][ct][:, h2 * 512:(h2 + 1) * 512],
                    start=(ct == 0),
                    stop=(ct == 7),
                )
                if j == 15:
                    nc.vector.tensor_scalar_add(
                        out=qk_dst[pi][ft][:, :],
                        in0=ps[:, :],
                        scalar1=qk_bias[pi][:, ft:ft + 1],
                    )
                    del qk_ps[key]

            # upfront: ft=0 for all projections, plus all fts of proj 2 (wky)
            for pi in range(3):
                emit_qk_half(pi, 0, 0)
                emit_qk_half(pi, 0, 1)
            for ft in range(1, 4):
                emit_qk_half(2, ft, 0)
                emit_qk_half(2, ft, 1)

            # remaining 6 ft-groups of Q/Kx ride the attention phase's spare
            # PE cycles, one matmul per (kt) group
            qk_fillers = {(t, qt): (qt, t + 1) for t in range(3) for qt in range(2)}

            # ---- attention (head pairs row-pack; both q-tiles share one
            #      psum tile so exp runs once per (kt, head)) ----
            oT = [big.tile([128, S], BF16, tag="big", name=f"oT{i}") for i in range(4)]

            def emit_finalize(t, qt, o_sb, recips):
                for hh in range(2):
                        i = hh
                        rd = recips[i]
                        bc_sb = spool.tile([DH, 512], FP32, tag="bc", name="bc_sb")
                        rd_bcast = bass.AP(
                            tensor=rd.tensor, offset=rd.offset,
                            ap=[[0, DH]] + [list(a) for a in rd.ap[1:]],
                        )
                        nc.gpsimd.dma_start(out=bc_sb[:, :], in_=rd_bcast)
                        nc.vector.tensor_mul(
                            out=oT[t][hh * 64:hh * 64 + DH, qt * 512:(qt + 1) * 512],
                            in0=o_sb[i][:, :],
                            in1=bc_sb[:, :],
                        )

            def emit_finalize_fast(t, qt, o_sb, rfs):
                # last-group variant: broadcast the reciprocal via a K=1
                # PE matmul into a just-freed PSUM bank instead of the DRAM
                # bounce — ~2us chain instead of ~7us, and the PE never
                # idles long enough for HAM to re-throttle.
                for hh in range(2):
                    bc_ps = ot_ps.tile([128, 512], FP32, tag="ot", name="bc_ps")
                    nc.tensor.matmul(
                        bc_ps[0:DH, :],
                        ones_f32[0:1, 0:DH],
                        rfs[hh][0:1, :],
                        start=True,
                        stop=True,
                    )
                    nc.vector.tensor_mul(
                        out=oT[t][hh * 64:hh * 64 + DH, qt * 512:(qt + 1) * 512],
                        in0=o_sb[hh][:, :],
                        in1=bc_ps[0:DH, :],
                    )

            pending = None
            for t in range(4):  # heads 2t, 2t+1
                for qt in range(2):
                    o_ps = [ot_ps.tile([128, 512], FP32, tag="ot", name=f"ops{i}")
                            for i in range(2)]  # per head of the pair
                    hist = []
                    for kt in range(16):
                        KT = KxT[t] if kt < 8 else KyT[t]
                        ks = (kt % 8) * 128
                        sc = mm_ps.tile([128, 1024], FP32, tag="mm", name="sc")
                        for hh in range(2):
                            nc.tensor.matmul(
                                sc[:, hh * 512:(hh + 1) * 512],
                                KT[hh * 64:(hh + 1) * 64, ks:ks + 128],
                                QT[t][hh * 64:(hh + 1) * 64, qt * 512:(qt + 1) * 512],
                                start=True,
                                stop=True,
                            )
                        p2 = ppool.tile([128, 1024], BF16, tag="p", name="p")
                        nc.scalar.activation(out=p2[:, :], in_=sc[:, :], func=EXP)
                        # AV trails the exp stream by 2 groups so the PE
                        # never head-of-line blocks on an in-flight exp
                        if kt >= 2:
                            for hh in range(2):
                                nc.tensor.matmul(
                                    o_ps[hh][0:DH + 1, :],
                                    V[kt - 2][:, 2 * t + hh, :],
                                    hist[kt - 2][:, hh * 512:(hh + 1) * 512],
                                    start=(kt == 2),
                                    stop=False,
                                )
                        # fillers after the AVs so their LDWEIGHTS hide
                        # behind the full-array AV matmuls; kt=0/1 carry two
                        # fillers each (no AVs there thanks to the lag-2
                        # structure) so the last filler lands 2 groups
                        # before the next sweep needs its output
                        if (t, qt) in qk_fillers:
                            fpi, fft = qk_fillers[(t, qt)]
                            if kt < 2:
                                emit_qk_single(fpi, fft, 2 * kt)
                                emit_qk_single(fpi, fft, 2 * kt + 1)
                            elif kt <= 13:
                                emit_qk_single(fpi, fft, kt + 2)
                        # previous group's finalize muls run mid-sweep when
                        # the DVE is otherwise idle, not at the boundary
                        if kt == 4 and pending is not None:
                            emit_finalize(*pending)
                            pending = None
                        hist.append(p2)
                    for kt_tail in (14, 15):
                        for hh in range(2):
                            nc.tensor.matmul(
                                o_ps[hh][0:DH + 1, :],
                                V[kt_tail][:, 2 * t + hh, :],
                                hist[kt_tail][:, hh * 512:(hh + 1) * 512],
                                start=False,
                                stop=(kt_tail == 15),
                            )
                    is_last = (t == 3 and qt == 1)
                    o_sb = []
                    rfs = []
                    recips = []
                    # den copies + reciprocals first so the (last-group) PE
                    # broadcast can start as early as possible; the previous
                    # group's finalize muls are emitted only after these
                    # copies so the o_ps banks free up without DVE
                    # head-of-line blocking
                    for i in range(2):
                        s2h = spool.tile([1, 512], FP32, tag="s2", name="s2h")
                        nc.vector.tensor_copy(out=s2h[:, :], in_=o_ps[i][DH:DH + 1, :])
                        rfh = spool.tile([1, 512], FP32, tag="recipf", name="rfh")
                        nc.vector.reciprocal_approx_fast(out=rfh[:, :], in_=s2h[:, :])
                        rfs.append(rfh)
                    for i in range(2):
                        ob = spool.tile([DH, 512], FP32, tag="osb", name="osb")
                        nc.vector.tensor_copy(out=ob[:, :], in_=o_ps[i][0:DH, :])
                        o_sb.append(ob)
                        if not is_last:
                            rd = dpool.tile([1, 512], FP32, name="rd")
                            nc.gpsimd.dma_start(out=rd[:, :], in_=rfs[i][0:1, :])
                            recips.append(rd)
                    assert pending is None
                    if is_last:
                        last_fast = (t, qt, o_sb, rfs)
                    else:
                        pending = (t, qt, o_sb, recips)
            if pending is not None:
                emit_finalize(*pending)
            emit_finalize_fast(*last_fast)

            # ---- out-projection (transposed domain [m, s]) ----
            # 3-deep software pipeline over the 8 m-tiles; ft0-2 first, ft3
            # (gated on the last head-pair's finalize) as late as possible.
            def op_mms(ps, mt, fts):
                for ft in fts:
                    for half in range(2):
                        nc.tensor.matmul(
                            ps[:, half * 512:(half + 1) * 512],
                            wo_sb[ft][:, mt * 128:(mt + 1) * 128],
                            oT[ft][:, half * 512:(half + 1) * 512],
                            start=(ft == 0),
                            stop=(ft == 3),
                        )

            def op_finish(ps, mt):
                osb = opool.tile([128, 1024], FP16, tag="osb2", name="osb2")
                # alternate the PSUM->SBUF cast between DVE and ACT so the
                # tail isn't serialized on one engine; split the store over
                # two DMA rings so the last transfer clears quickly
                if mt % 2 == 0:
                    nc.vector.tensor_copy(out=osb[:, :], in_=ps[:, :])
                    eng = nc.sync
                else:
                    nc.scalar.copy(out=osb[:, :], in_=ps[:, :])
                    eng = nc.scalar
                eng.dma_start(out=outT[mt * 128:(mt + 1) * 128, 0:512],
                              in_=osb[:, 0:512])
                eng.dma_start(out=outT[mt * 128:(mt + 1) * 128, 512:1024],
                              in_=osb[:, 512:1024])

            op_tiles = {}
            for mt in range(3):
                op_tiles[mt] = mm_ps.tile([128, 1024], FP32, tag="mm", name=f"obs{mt}")
                op_mms(op_tiles[mt], mt, range(3))
            for mt in range(8):
                op_mms(op_tiles[mt], mt, [3])
                op_finish(op_tiles[mt], mt)
                nxt = mt + 3
                if nxt < 8:
                    op_tiles[nxt] = mm_ps.tile([128, 1024], FP32, tag="mm", name=f"obs{nxt}")
                    op_mms(op_tiles[nxt], nxt, range(3))

    # populate .instr bytes for extended-inst InstISA subclasses (the
    # custom-DVE reciprocal) — raw Bass skips this pass and the NEFF
    # compiler errors with "ISA wrong length" without it
    mybir.codegen_inst_isa_subclasses(nc)
    _spill_excess_waits(nc)
    return nc


_NC = None


def _get_program():
    global _NC
    if _NC is None:
        _NC = _build_program()
    return _NC


# ---------------------------------------------------------------------------
# host wrapper
# ---------------------------------------------------------------------------

def _prep_in_maps(x, y, W_Kx, b_Kx, W_Qx, b_Qx, W_Vx, b_Vx, W_Ky, b_Ky,
                  W_Vy, b_Vy, W_out, b_out):
    f32 = np.float32
    bf16 = ml_dtypes.bfloat16
    in_maps = []
    for c in range(NCORES):
        b = c // 2
        g = c % 2
        gs = slice(FS * g, FS * (g + 1))
        m = {
            "xT": np.ascontiguousarray(np.asarray(x[b], f32).T).astype(bf16),
            "yT": np.ascontiguousarray(np.asarray(y[b], f32).T).astype(bf16),
            "wq": np.ascontiguousarray((np.asarray(W_Qx, f32)[gs, :] / 8.0).T).astype(bf16),
            "wkx": np.ascontiguousarray(np.asarray(W_Kx, f32)[gs, :].T).astype(bf16),
            "wky": np.ascontiguousarray(np.asarray(W_Ky, f32)[gs, :].T).astype(bf16),
            "wvx": np.ascontiguousarray(np.asarray(W_Vx, f32)[gs, :].T).astype(bf16),
            "wvy": np.ascontiguousarray(np.asarray(W_Vy, f32)[gs, :].T).astype(bf16),
            "wo": np.ascontiguousarray(np.asarray(W_out, f32)[:, gs].T).astype(bf16),
            "bq": np.ascontiguousarray(
                (np.asarray(b_Qx, f32)[gs] / 8.0).reshape(4, 128).T),
            "bkx": np.ascontiguousarray(np.asarray(b_Kx, f32)[gs].reshape(4, 128).T),
            "bky": np.ascontiguousarray(np.asarray(b_Ky, f32)[gs].reshape(4, 128).T),
            "bvx_bc": np.ascontiguousarray(np.asarray(b_Vx, f32)[gs].reshape(1, FS)),
            "bvy_bc": np.ascontiguousarray(np.asarray(b_Vy, f32)[gs].reshape(1, FS)),
        }
        in_maps.append(m)
    return in_maps


def _assemble(results, b_out):
    B = 4
    out = np.empty((B, S, DIM), np.float32)
    bo = np.asarray(b_out, np.float32)
    for b in range(B):
        acc = (results[2 * b]["outT"].astype(np.float32)
               + results[2 * b + 1]["outT"].astype(np.float32))
        out[b] = acc.T + bo
    return out


def kernel(**inputs):
    nc = _get_program()
    in_maps = _prep_in_maps(**inputs)
    last_err = None
    for _attempt in range(3):
        try:
            res = run_bass_kernel_spmd(nc, in_maps, core_ids=list(range(NCORES)))
            return _assemble(res.results, inputs["b_out"])
        except Exception as e:  # transient NRT_EXEC_UNIT_UNRECOVERABLE after fresh compile
            last_err = e
            import time as _time
            _time.sleep(2.0)
    raise last_err


def kernel_traced(trace_cores=None, **inputs):
    """Same as kernel() but returns (out, BassKernelResults) with NTFF trace."""
    _register_ntff_hook()
    nc = _get_program()
    in_maps = _prep_in_maps(**inputs)
    res = run_bass_kernel_spmd(
        nc, in_maps, core_ids=list(range(NCORES)), trace=True,
        trace_cores=trace_cores or [0],
    )
    return _assemble(res.results, inputs["b_out"]), res
